# revision 1
# baseline (speedup 1.0000x reference)
"""GAT 2-layer kernel for TRN2, 8 NeuronCores (self-contained).

Strategy:
- dst-shard: core c owns nodes [c*12500, (c+1)*12500).
- Dense phases (x@W, h1@W2) replicated on all cores; folded weights give
  per-node [h | as | ad] in one matmul.
- Edge phase per core: 4 src-chunks (25000 nodes each, so dma_gather's int16
  indices reach every row), per-chunk degree-bucketed padded CSR over dst.
  One dma_gather per tile pulls rows [h | as_hi | as_lo] (bf16, 512B L1 /
  256B L2). exp(leaky(as+ad)) and weighted feature sums on DVE/ACT.
- Per-chunk partials [featsum | denom] -> DRAM staging (bf16).
- Merge pass (natural node order): dma_gather the 4 partial rows per node,
  combine, per-head normalize, bias (+relu on L1).
- L1->L2: PE-transpose out1 -> h1T shard -> AllGather -> replicated dense2.
"""
import sys
sys.path.insert(0, "/opt/trn_rl_repo")
import numpy as np
import ml_dtypes

import concourse.bass as bass
import concourse.bacc as bacc
import concourse.tile as tile
from concourse import mybir
from concourse.library_config import mlp as mlp_lib


def make_runner(nc, n_cores):
    """PJRT runner: returns run_fn(in_maps, repeats) -> (results, best_time_s)."""
    import time
    import jax
    from jax.sharding import Mesh, PartitionSpec, NamedSharding
    from jax.experimental.shard_map import shard_map
    from concourse.bass2jax import (_bass_exec_p, install_neuronx_cc_hook,
                                    partition_id_tensor)
    install_neuronx_cc_hook()
    partition_name = nc.partition_id_tensor.name if nc.partition_id_tensor else None
    in_names, out_names, out_avals, zero_outs = [], [], [], []
    for alloc in nc.m.functions[0].allocations:
        if not isinstance(alloc, mybir.MemoryLocationSet):
            continue
        if not alloc.memorylocations:
            continue
        name = alloc.memorylocations[0].name
        if alloc.kind == "ExternalInput":
            if name != partition_name:
                in_names.append(name)
        elif alloc.kind == "ExternalOutput":
            out_names.append(name)
            shape = tuple(alloc.tensor_shape)
            dtype = mybir.dt.np(alloc.dtype)
            out_avals.append(jax.core.ShapedArray(shape, dtype))
            zero_outs.append(np.zeros(shape, dtype))
    n_params = len(in_names)
    n_outs = len(out_avals)
    all_in_names = list(in_names) + list(out_names)
    if partition_name is not None:
        all_in_names.append(partition_name)

    def _body(*args):
        operands = list(args)
        if partition_name is not None:
            operands.append(partition_id_tensor())
        return tuple(_bass_exec_p.bind(
            *operands, out_avals=tuple(out_avals), in_names=tuple(all_in_names),
            out_names=tuple(out_names), lowering_input_output_aliases=(),
            sim_require_finite=False, sim_require_nnan=False, nc=nc))

    devices = jax.devices()[:n_cores]
    mesh = Mesh(np.asarray(devices), ("core",))
    in_specs = (PartitionSpec("core"),) * (n_params + n_outs)
    out_specs = (PartitionSpec("core"),) * n_outs
    donate = tuple(range(n_params, n_params + n_outs))
    sharded = jax.jit(
        shard_map(_body, mesh=mesh, in_specs=in_specs, out_specs=out_specs,
                  check_rep=False),
        donate_argnums=donate, keep_unused=True)

    def run_fn(in_maps, repeats=1):
        per_core = [[np.asarray(m[name]) for name in in_names] for m in in_maps]
        concat_in = [np.concatenate([per_core[c][i] for c in range(n_cores)], 0)
                     for i in range(n_params)]
        sharding = NamedSharding(mesh, PartitionSpec("core"))
        dev_in = [jax.device_put(a, sharding) for a in concat_in]
        for a in dev_in:
            a.block_until_ready()
        times, out_arrs = [], None
        for _ in range(repeats):
            concat_zeros = [jax.device_put(
                np.zeros((n_cores * z.shape[0], *z.shape[1:]), z.dtype), sharding)
                for z in zero_outs]
            for z in concat_zeros:
                z.block_until_ready()
            t0 = time.perf_counter()
            out_arrs = sharded(*dev_in, *concat_zeros)
            for o in out_arrs:
                o.block_until_ready()
            times.append(time.perf_counter() - t0)
        results = [
            {name: np.asarray(out_arrs[i]).reshape(n_cores, *out_avals[i].shape)[c]
             for i, name in enumerate(out_names)}
            for c in range(n_cores)]
        return results, min(times)

    return run_fn

F32 = mybir.dt.float32
BF16 = mybir.dt.bfloat16
I16 = mybir.dt.int16
BF = ml_dtypes.bfloat16

NCORES = 8
N = 100000
IN_DIM = 128
HID = 32
OUT_DIM = 16
NSH = N // NCORES            # 12500
NT = 98                      # merge tiles per core
NSHP = NT * 128              # 12544
CH1 = 25000                  # table1 nodes per chunk
NCH = 4
CH1R = CH1 + 1               # +pad row
R2 = NCORES * NSHP           # 100352
CH2 = R2 // NCH              # 25088 (= 2 padded cores)
CH2R = CH2 + 1
BUCKETS = (1, 2, 3, 4, 6, 8, 12, 24)
NEG = -1.0e30
EPS = 1e-16
NQ = 4                       # SWDGE queues
MB = 8                       # merge batch (tiles)

AluOp = mybir.AluOpType
ActFn = mybir.ActivationFunctionType
Axis = mybir.AxisListType


def _colgroups(D):
    out = []
    c = 0
    while c < D:
        w = min(8, D - c)
        out.append((c, w))
        c += w
    return out


def plan_segments(T):
    """Shared host/device plan. Returns:
    calls: list of (k, bi, t, colbase, w, stream_off) slot-gather calls
    rowbase: dict (k, bi, t) -> grid row base within chunk k
    grid_rows: [NCH] rows per chunk
    ad_off: dict (k, gridtile) handled implicitly (sequential)
    stream_len: total slot stream length
    """
    calls = []
    rowbase = {}
    grid_rows = []
    off = 0
    for k in range(NCH):
        rb = 0
        for bi, D in enumerate(BUCKETS):
            for t in range(int(T[k][bi])):
                rowbase[(k, bi, t)] = rb
                for (c0, w) in _colgroups(D):
                    calls.append((k, bi, t, c0, w, off))
                    off += 128 * w
                rb += 128
        grid_rows.append(rb)
    return calls, rowbase, grid_rows, off


def _wrap_idx(flat):
    n = len(flat)
    assert n % 16 == 0
    w = np.asarray(flat, np.int16).reshape(n // 16, 16).T
    return np.ascontiguousarray(np.tile(w, (8, 1)))


def fold(W, a):
    Hh, F = a.shape
    w = np.zeros((W.shape[0], Hh), np.float32)
    for h in range(Hh):
        w[:, h] = W[:, h * F:(h + 1) * F] @ a[h]
    return w


def host_prep(x, edge_index, W1, a1_src, a1_dst, b1, W2, a2_src, a2_dst, b2):
    x = np.asarray(x, np.float32)
    ei = np.asarray(edge_index)
    src = ei[0].astype(np.int64)
    dst = ei[1].astype(np.int64)
    W1 = np.asarray(W1, np.float32)
    W2 = np.asarray(W2, np.float32)
    Waug1 = np.concatenate([W1, fold(W1, np.asarray(a1_src, np.float32)),
                            fold(W1, np.asarray(a1_dst, np.float32))], 1)
    Waug2 = np.concatenate([W2, fold(W2, np.asarray(a2_src, np.float32)),
                            fold(W2, np.asarray(a2_dst, np.float32))], 1)
    xT = np.ascontiguousarray(x.T)

    core_of = dst // NSH
    # ---- per-core, per-chunk CSR ----
    pc = []  # [core][chunk] = (deg, sorted_src_by_dst, starts)
    for c in range(NCORES):
        m = core_of == c
        s_c, d_c = src[m], dst[m] - c * NSH
        ch = s_c // CH1
        info = []
        for k in range(NCH):
            mk = ch == k
            sk, dk = s_c[mk], d_c[mk]
            deg = np.bincount(dk, minlength=NSH)
            order = np.argsort(dk, kind="stable")
            sk = sk[order]
            starts = np.zeros(NSH + 1, np.int64)
            np.cumsum(deg, out=starts[1:])
            info.append((deg, sk, starts))
        pc.append(info)

    # shared tile counts
    T = [[0] * len(BUCKETS) for _ in range(NCH)]
    for c in range(NCORES):
        for k in range(NCH):
            deg = pc[c][k][0]
            for bi, D in enumerate(BUCKETS):
                lo = BUCKETS[bi - 1] if bi else 0
                nb = int(((deg > lo) & (deg <= D)).sum())
                T[k][bi] = max(T[k][bi], (nb + 127) // 128)
            assert deg.max(initial=0) <= BUCKETS[-1], f"deg max {deg.max()}"
    calls, rowbase, grid_rows, stream_len = plan_segments(T)

    b1rep = np.tile(np.asarray(b1, np.float32)[None, :], (128, 1))
    b2rep = np.tile(np.asarray(b2, np.float32)[None, :], (128, 1))
    pad1 = np.zeros((1, 256), BF); pad1[0, 128:132] = NEG
    pad2 = np.zeros((1, 128), BF); pad2[0, 64:68] = NEG
    z256 = np.zeros((1, 256), BF)

    in_maps = []
    for c in range(NCORES):
        slot_nodes = []   # per chunk: grid row -> node (or -1)
        for k in range(NCH):
            gr = grid_rows[k]
            deg, sk, starts = pc[c][k]
            nodes_of = np.full(gr, -1, np.int64)
            for bi, D in enumerate(BUCKETS):
                lo = BUCKETS[bi - 1] if bi else 0
                nd = np.where((deg > lo) & (deg <= D))[0]
                rb = rowbase[(k, bi, 0)] if T[k][bi] else 0
                nodes_of[rb:rb + len(nd)] = nd
            slot_nodes.append(nodes_of)

        s1 = np.full(stream_len, CH1, np.int64)     # pad -> table1 chunk pad row
        s2 = np.full(stream_len, CH2, np.int64)     # pad -> table2 chunk pad row
        for (k, bi, t, c0, w, off) in calls:
            D = BUCKETS[bi]
            rb = rowbase[(k, bi, t)]
            deg, sk, starts = pc[c][k]
            nodes = slot_nodes[k][rb:rb + 128]
            j = off
            for d in range(c0, c0 + w):
                for p in range(128):
                    nd = nodes[p]
                    if nd >= 0 and d < starts[nd + 1] - starts[nd]:
                        s = sk[starts[nd] + d]
                        s1[j] = s % CH1
                        s2[j] = (s // NSH % 2) * NSHP + s % NSH
                    j += 1
        slot1w = _wrap_idx(s1)
        slot2w = _wrap_idx(s2)

        # ad idx: per (k, gridtile) 128 local dst ids (pad -> 0)
        adix = []
        for k in range(NCH):
            nd = slot_nodes[k]
            adix.append(np.where(nd >= 0, nd, 0))
        adw = _wrap_idx(np.concatenate(adix)) if stream_len else None

        # merge idx: per chunk, per natural node (padded to NSHP): grid row or zero-row
        mrg = []
        for k in range(NCH):
            deg = pc[c][k][0]
            pos = np.full(NSHP, grid_rows[k], np.int64)  # zero row
            nd = slot_nodes[k]
            real = nd >= 0
            pos[nd[real]] = np.nonzero(real)[0]
            mrg.append(pos)
        mrgw = _wrap_idx(np.concatenate(mrg))

        in_maps.append(dict(
            xT=xT, Waug1=Waug1, Waug2=Waug2.astype(BF),
            b1rep=b1rep, b2rep=b2rep, pad1=pad1, pad2=pad2, z256=z256,
            slot1w=slot1w, slot2w=slot2w, adw=adw, mrgw=mrgw,
        ))
    meta = dict(T=T, calls=calls, rowbase=rowbase, grid_rows=grid_rows,
                stream_len=stream_len)
    return in_maps, meta


def vap(t, off, dims):
    a = t[:]
    return bass.AP(a.tensor, a.offset + off, [list(a.ap[0])] + [list(d) for d in dims])


def build_nc(meta):
    T = meta["T"]
    calls = meta["calls"]
    rowbase = meta["rowbase"]
    grid_rows = meta["grid_rows"]
    stream_len = meta["stream_len"]
    SW = stream_len // 16
    ADL = sum(grid_rows)            # ad idx count (128 per grid tile) / 128 * 128
    AW = ADL // 16
    MW = (NCH * NSHP) // 16

    nc = bacc.Bacc("TRN2", target_bir_lowering=False, num_swdge_queues=NQ)
    dp = nc.declare_dram_parameter
    xT = dp("xT", [IN_DIM, N], F32, isOutput=False)
    Waug1 = dp("Waug1", [128, 136], F32, isOutput=False)
    Waug2 = dp("Waug2", [32, 72], BF16, isOutput=False)
    b1rep = dp("b1rep", [128, HID], F32, isOutput=False)
    b2rep = dp("b2rep", [128, OUT_DIM], F32, isOutput=False)
    pad1 = dp("pad1", [1, 256], BF16, isOutput=False)
    pad2 = dp("pad2", [1, 128], BF16, isOutput=False)
    z256 = dp("z256", [1, 256], BF16, isOutput=False)
    slot1w = dp("slot1w", [128, SW], I16, isOutput=False)
    slot2w = dp("slot2w", [128, SW], I16, isOutput=False)
    adw = dp("adw", [128, AW], I16, isOutput=False)
    mrgw = dp("mrgw", [128, MW], I16, isOutput=False)
    out2 = dp("out2", [NSHP, OUT_DIM], F32, isOutput=True)

    table1 = nc.dram_tensor("table1", [NCH * CH1R, 256], BF16)
    table2 = nc.dram_tensor("table2", [NCH * CH2R, 128], BF16)
    ad1nat = nc.dram_tensor("ad1nat", [N, 4], F32)
    ad2nat = nc.dram_tensor("ad2nat", [R2, 4], F32)
    ad1c = nc.dram_tensor("ad1c", [NSHP, 64], F32)
    ad2c = nc.dram_tensor("ad2c", [NSHP, 64], F32)
    stg1 = [nc.dram_tensor(f"stg1_{k}", [grid_rows[k] + 1, 256], BF16)
            for k in range(NCH)]
    stg2 = [nc.dram_tensor(f"stg2_{k}", [grid_rows[k] + 1, 128], BF16)
            for k in range(NCH)]
    h1T_sh = nc.dram_tensor("h1T_sh", [32, NSHP], BF16)
    h1T_all = nc.dram_tensor("h1T_all", [NCORES, 32, NSHP], BF16,
                             addr_space="Shared")

    qn = [0]
    def nextq():
        qn[0] = (qn[0] + 1) % NQ
        return qn[0]

    with tile.TileContext(nc) as tc:
        nc.gpsimd.load_library(mlp_lib)

        # ---------- consts / pads ----------
        with tc.tile_pool(name="konst", bufs=1) as kp:
            w1sb = kp.tile([128, 136], F32)
            nc.sync.dma_start(out=w1sb[:], in_=Waug1[:, :])
            w2sb = kp.tile([32, 72], BF16)
            nc.sync.dma_start(out=w2sb[:], in_=Waug2[:, :])
            b1sb = kp.tile([128, HID], F32)
            nc.sync.dma_start(out=b1sb[:], in_=b1rep[:, :])
            b2sb = kp.tile([128, OUT_DIM], F32)
            nc.sync.dma_start(out=b2sb[:], in_=b2rep[:, :])
            for k in range(NCH):
                nc.sync.dma_start(out=table1[k * CH1R + CH1, :], in_=pad1[0, :])
                nc.sync.dma_start(out=table2[k * CH2R + CH2, :], in_=pad2[0, :])
                nc.sync.dma_start(out=stg1[k][grid_rows[k], :], in_=z256[0, :])
                nc.sync.dma_start(out=stg2[k][grid_rows[k], :], in_=z256[0, :128])

            # ---------- dense1 ----------
            with (tc.tile_pool(name="d1", bufs=4) as dpool,
                  tc.tile_pool(name="d1p", bufs=4, space="PSUM") as dps):
                for t in range((N + 127) // 128):
                    n0 = 128 * t
                    nn = min(128, N - n0)
                    xm = dpool.tile([128, 128], F32, tag="xm")
                    nc.sync.dma_start(out=xm[:, 0:nn], in_=bass.AP(
                        xT[:, :].tensor, n0, [[N, 128], [1, nn]]))
                    ps = dps.tile([128, 136], F32, tag="ps")
                    nc.tensor.matmul(out=ps[0:nn, :], lhsT=xm[:, 0:nn],
                                     rhs=w1sb[:], start=True, stop=True)
                    hrow = dpool.tile([128, 256], BF16, tag="hrow")
                    nc.vector.tensor_copy(out=hrow[0:nn, 0:132],
                                          in_=ps[0:nn, 0:132])
                    hi32 = dpool.tile([128, 4], F32, tag="hi32")
                    nc.any.tensor_copy(out=hi32[0:nn], in_=hrow[0:nn, 128:132])
                    nc.vector.tensor_sub(out=hrow[0:nn, 132:136],
                                         in0=ps[0:nn, 128:132], in1=hi32[0:nn])
                    adsb = dpool.tile([128, 4], F32, tag="adsb")
                    nc.any.tensor_copy(out=adsb[0:nn], in_=ps[0:nn, 132:136])
                    nc.sync.dma_start(out=ad1nat[n0:n0 + nn, :],
                                      in_=adsb[0:nn])
                    # table1 rows with chunk shift (may straddle chunk boundary)
                    k0, k1 = n0 // CH1, (n0 + nn - 1) // CH1
                    if k0 == k1:
                        r0 = k0 * CH1R + (n0 - k0 * CH1)
                        nc.sync.dma_start(
                            out=bass.AP(table1[:, :].tensor, r0 * 256,
                                        [[256, nn], [1, 136]]),
                            in_=hrow[0:nn, 0:136])
                    else:
                        nsplit = k1 * CH1 - n0
                        r0 = k0 * CH1R + (n0 - k0 * CH1)
                        nc.sync.dma_start(
                            out=bass.AP(table1[:, :].tensor, r0 * 256,
                                        [[256, nsplit], [1, 136]]),
                            in_=hrow[0:nsplit, 0:136])
                        r1 = k1 * CH1R
                        nc.sync.dma_start(
                            out=bass.AP(table1[:, :].tensor, r1 * 256,
                                        [[256, nn - nsplit], [1, 136]]),
                            in_=hrow[nsplit:nn, 0:136])

            # ---------- repack ad1: own 12500 rows -> ad1c [NSHP, 64] ----------
            pid = nc.gpsimd.partition_id()
            with tc.tile_pool(name="rp", bufs=2) as rp:
                adt = rp.tile([128, 98 * 4], F32, tag="adt")
                nc.gpsimd.memset(adt[:], 0.0)
                base = pid * NSH
                nc.gpsimd.dma_start(
                    out=vap(adt, 0, [[4, 97], [1, 4]]),
                    in_=ad1nat[bass.ds(base, 12416), :]
                        .rearrange("(t p) f -> p t f", p=128))
                nc.gpsimd.dma_start(
                    out=bass.AP(adt[:].tensor, adt[:].offset + 97 * 4,
                                [[list(adt[:].ap[0])[0], 84], [1, 4]]),
                    in_=ad1nat[bass.ds(base + 12416, 84), :])
                nc.sync.dma_start(
                    out=bass.AP(ad1c[:, :].tensor, 0,
                                [[64, 128], [64 * 128, 98], [1, 4]]),
                    in_=adt[:])

            # ---------- edge pass L1 ----------
            _edge_pass(nc, tc, meta, layer=1, slotw=slot1w, adw=adw,
                       table=table1, stg=stg1, ad_core=ad1c, nextq=nextq)

            # ---------- merge L1 -> h1T ----------
            _merge_pass(nc, tc, meta, layer=1, mrgw=mrgw, stg=stg1,
                        bsb=b1sb, out2=None, h1T_sh=h1T_sh, nextq=nextq)

            # ---------- allgather ----------
            nc.gpsimd.collective_compute(
                "AllGather", AluOp.bypass,
                replica_groups=[list(range(NCORES))],
                ins=[h1T_sh[:, :]], outs=[h1T_all[:, :, :]])

            # ---------- dense2 ----------
            with (tc.tile_pool(name="d2", bufs=4) as dpool,
                  tc.tile_pool(name="d2p", bufs=4, space="PSUM") as dps):
                for q in range(R2 // 128):
                    lh = dpool.tile([32, 128], BF16, tag="lh")
                    cc, tt = q // NT, q % NT
                    nc.sync.dma_start(out=lh[:], in_=bass.AP(
                        h1T_all[:, :, :].tensor, cc * 32 * NSHP + 128 * tt,
                        [[NSHP, 32], [1, 128]]))
                    ps = dps.tile([128, 72], F32, tag="ps2")
                    nc.tensor.matmul(out=ps[:], lhsT=lh[:], rhs=w2sb[:],
                                     start=True, stop=True)
                    h2row = dpool.tile([128, 128], BF16, tag="h2row")
                    nc.vector.tensor_copy(out=h2row[:, 0:68], in_=ps[:, 0:68])
                    hi32 = dpool.tile([128, 4], F32, tag="hi232")
                    nc.any.tensor_copy(out=hi32[:], in_=h2row[:, 64:68])
                    nc.vector.tensor_sub(out=h2row[:, 68:72],
                                         in0=ps[:, 64:68], in1=hi32[:])
                    adsb = dpool.tile([128, 4], F32, tag="adsb2")
                    nc.any.tensor_copy(out=adsb[:], in_=ps[:, 68:72])
                    nc.sync.dma_start(out=ad2nat[128 * q:128 * (q + 1), :],
                                      in_=adsb[:])
                    r0 = q * 128 + (q * 128) // CH2 * 1
                    nc.sync.dma_start(
                        out=bass.AP(table2[:, :].tensor, r0 * 128,
                                    [[128, 128], [1, 72]]),
                        in_=h2row[:, 0:72])

            # ---------- repack ad2 ----------
            with tc.tile_pool(name="rp2", bufs=2) as rp:
                adt = rp.tile([128, 98 * 4], F32, tag="adt2")
                base2 = pid * NSHP
                nc.gpsimd.dma_start(
                    out=vap(adt, 0, [[4, 98], [1, 4]]),
                    in_=ad2nat[bass.ds(base2, NSHP), :]
                        .rearrange("(t p) f -> p t f", p=128))
                nc.sync.dma_start(
                    out=bass.AP(ad2c[:, :].tensor, 0,
                                [[64, 128], [64 * 128, 98], [1, 4]]),
                    in_=adt[:])

            # ---------- edge pass L2 ----------
            _edge_pass(nc, tc, meta, layer=2, slotw=slot2w, adw=adw,
                       table=table2, stg=stg2, ad_core=ad2c, nextq=nextq)

            # ---------- merge L2 -> out2 ----------
            _merge_pass(nc, tc, meta, layer=2, mrgw=mrgw, stg=stg2,
                        bsb=b2sb, out2=out2, h1T_sh=None, nextq=nextq)

    nc.finalize()
    return nc


def _edge_pass(nc, tc, meta, layer, slotw, adw, table, stg, ad_core, nextq):
    calls = meta["calls"]
    rowbase = meta["rowbase"]
    grid_rows = meta["grid_rows"]
    RW = 256 if layer == 1 else 128       # table row elems (bf16)
    FD = 128 if layer == 1 else 64        # feature elems
    SW = meta["stream_len"] // 16
    AW = sum(grid_rows) // 16

    # stream offset of each bucket's first slot (buckets are contiguous)
    KR = CH1R if layer == 1 else CH2R
    bstart = {}
    for (k, bi, t, c0, w, off) in calls:
        bstart.setdefault((k, bi), off)

    with (tc.tile_pool(name=f"eidx{layer}", bufs=1) as ip,
          tc.tile_pool(name=f"eg{layer}", bufs=3) as gp,
          tc.tile_pool(name=f"ea{layer}", bufs=2) as ap_pool,
          tc.tile_pool(name=f"ew{layer}", bufs=2) as wp):
        sidx = ip.tile([128, SW], I16, tag="sidx")
        nc.sync.dma_start(out=sidx[:], in_=slotw[:, :])
        aidx = ip.tile([128, AW], I16, tag="aidx")
        nc.sync.dma_start(out=aidx[:], in_=adw[:, :])

        abase = 0
        for k in range(NCH):
            # per-chunk upfront ad gather (all grid tiles of this chunk)
            TK = grid_rows[k] // 128
            ADG = ap_pool.tile([128, TK, 64], F32, tag="ADG")
            na = TK * 128
            o = 0
            while o < na:
                nb = min(1024, na - o)
                nc.gpsimd.dma_gather(
                    ADG[:, o // 128:(o + nb) // 128, :], ad_core[:, :],
                    aidx[:, (abase + o) // 16:(abase + o + nb) // 16],
                    nb, nb, 64, queue_num=nextq())
                o += nb
            abase += na

            tk = 0  # tile index within chunk
            for bi, D in enumerate(BUCKETS):
                T = int(meta["T"][k][bi])
                if T == 0:
                    continue
                ncols = T * D
                G = gp.tile([128, ncols, RW], BF16, tag="G")
                off = bstart[(k, bi)]
                c = 0
                while c < ncols:
                    w = min(8, ncols - c)
                    nc.gpsimd.dma_gather(
                        G[:, c:c + w, :], table[k * KR:(k + 1) * KR, :],
                        sidx[:, (off + 128 * c) // 16:(off + 128 * (c + w)) // 16],
                        128 * w, 128 * w, RW, queue_num=nextq())
                    c += w
                # bucket-level scalar chain
                e = wp.tile([128, ncols * 4], F32, tag="e")
                nc.vector.tensor_tensor(
                    out=e[:], in0=vap(G, FD, [[RW, ncols], [1, 4]]),
                    in1=vap(G, FD + 4, [[RW, ncols], [1, 4]]), op=AluOp.add)
                nc.vector.tensor_tensor(
                    out=e[:], in0=e[:],
                    in1=bass.AP(ADG[:].tensor,
                                ADG[:].offset + tk * 64,
                                [list(ADG[:].ap[0]), [64, T], [0, D], [1, 4]]),
                    op=AluOp.add)
                nc.vector.scalar_tensor_tensor(
                    out=e[:], in0=e[:], scalar=0.2, in1=e[:],
                    op0=AluOp.mult, op1=AluOp.max)
                nc.scalar.activation(out=e[:], in_=e[:], func=ActFn.Exp)
                den = wp.tile([128, T * 4], F32, tag="den")
                nc.vector.tensor_reduce(
                    out=den[:, 0:T * 4],
                    in_=vap(e, 0, [[4 * D, T], [1, 4], [4, D]]),
                    axis=Axis.X, op=AluOp.add)
                for t in range(T):
                    val = wp.tile([128, D * FD], F32, tag="val")
                    nc.vector.tensor_tensor(
                        out=val[:],
                        in0=vap(G, t * D * RW,
                                [[RW, D], [FD // 4, 4], [1, FD // 4]]),
                        in1=vap(e, t * D * 4, [[4, D], [1, 4], [0, FD // 4]]),
                        op=AluOp.mult)
                    fs = wp.tile([128, FD], F32, tag="fs")
                    nc.vector.tensor_reduce(
                        out=fs[:], in_=vap(val, 0, [[1, FD], [FD, D]]),
                        axis=Axis.X, op=AluOp.add)
                    so = wp.tile([128, RW], BF16, tag="so")
                    nc.vector.tensor_copy(out=so[:, 0:FD], in_=fs[:])
                    nc.vector.tensor_copy(out=so[:, FD:FD + 4],
                                          in_=den[:, t * 4:t * 4 + 4])
                    rb = rowbase[(k, bi, t)]
                    nc.sync.dma_start(
                        out=bass.AP(stg[k][:, :].tensor, rb * RW,
                                    [[RW, 128], [1, RW]]),
                        in_=so[:])
                tk += T


def _merge_pass(nc, tc, meta, layer, mrgw, stg, bsb, out2, h1T_sh, nextq):
    grid_rows = meta["grid_rows"]
    RW = 256 if layer == 1 else 128
    FD = 128 if layer == 1 else 64
    OD = HID if layer == 1 else OUT_DIM
    MW = (NCH * NSHP) // 16

    batches = []
    mt = 0
    while mt < NT:
        nb = min(MB, NT - mt)
        batches.append((mt, nb))
        mt += nb

    with (tc.tile_pool(name=f"midx{layer}", bufs=1) as ip,
          tc.tile_pool(name=f"mg{layer}", bufs=2) as gp,
          tc.tile_pool(name=f"mw{layer}", bufs=2) as wp,
          tc.tile_pool(name=f"mp{layer}", bufs=2, space="PSUM") as pp):
        midx = ip.tile([128, MW], I16, tag="midx")
        nc.sync.dma_start(out=midx[:], in_=mrgw[:, :])
        if layer == 1:
            from concourse.masks import make_identity
            ident = ip.tile([128, 128], F32, tag="ident")
            make_identity(nc, ident[:])

        for (mt, nb) in batches:
            Gs = []
            for k in range(NCH):
                Gk = gp.tile([128, MB, RW], BF16, tag=f"MG{k}")
                ioff = k * NSHP + mt * 128
                nc.gpsimd.dma_gather(
                    Gk[:, 0:nb, :], stg[k][:, :],
                    midx[:, ioff // 16:(ioff + nb * 128) // 16],
                    nb * 128, nb * 128, RW, queue_num=nextq())
                Gs.append(Gk)
            W = FD + 4
            s01 = wp.tile([128, MB * W], F32, tag="s01")
            nc.vector.tensor_tensor(
                out=vap(s01, 0, [[W, nb], [1, W]]),
                in0=vap(Gs[0], 0, [[RW, nb], [1, W]]),
                in1=vap(Gs[1], 0, [[RW, nb], [1, W]]), op=AluOp.add)
            s23 = wp.tile([128, MB * W], F32, tag="s23")
            nc.vector.tensor_tensor(
                out=vap(s23, 0, [[W, nb], [1, W]]),
                in0=vap(Gs[2], 0, [[RW, nb], [1, W]]),
                in1=vap(Gs[3], 0, [[RW, nb], [1, W]]), op=AluOp.add)
            nc.vector.tensor_tensor(
                out=vap(s01, 0, [[W, nb], [1, W]]),
                in0=vap(s01, 0, [[W, nb], [1, W]]),
                in1=vap(s23, 0, [[W, nb], [1, W]]), op=AluOp.add)
            rec = wp.tile([128, MB * 4], F32, tag="rec")
            nc.vector.tensor_scalar_add(
                out=vap(rec, 0, [[4, nb], [1, 4]]),
                in0=vap(s01, FD, [[W, nb], [1, 4]]), scalar1=EPS)
            nc.vector.reciprocal(out=rec[:, 0:nb * 4], in_=rec[:, 0:nb * 4])
            nc.vector.tensor_scalar_mul(out=rec[:, 0:nb * 4],
                                        in0=rec[:, 0:nb * 4], scalar1=0.25)
            sc = wp.tile([128, MB * FD], F32, tag="sc")
            nc.vector.tensor_tensor(
                out=vap(sc, 0, [[FD, nb], [FD // 4, 4], [1, FD // 4]]),
                in0=vap(s01, 0, [[W, nb], [FD // 4, 4], [1, FD // 4]]),
                in1=vap(rec, 0, [[4, nb], [1, 4], [0, FD // 4]]),
                op=AluOp.mult)
            hs = wp.tile([128, MB * OD], F32, tag="hs")
            nc.vector.tensor_reduce(
                out=vap(hs, 0, [[OD, nb], [1, OD]]),
                in_=vap(sc, 0, [[FD, nb], [1, OD], [OD, 4]]),
                axis=Axis.X, op=AluOp.add)
            nc.vector.tensor_tensor(
                out=vap(hs, 0, [[OD, nb], [1, OD]]),
                in0=vap(hs, 0, [[OD, nb], [1, OD]]),
                in1=vap(bsb, 0, [[0, nb], [1, OD]]), op=AluOp.add)
            if layer == 1:
                nc.scalar.activation(out=hs[:, 0:nb * OD], in_=hs[:, 0:nb * OD],
                                     func=ActFn.Relu)
                for ti in range(nb):
                    psT = pp.tile([32, 128], F32, tag="psT")
                    nc.tensor.transpose(out=psT[:],
                                        in_=hs[:, ti * OD:(ti + 1) * OD],
                                        identity=ident[:])
                    hsb = wp.tile([32, 128], BF16, tag="hsb")
                    nc.vector.tensor_copy(out=hsb[:], in_=psT[:])
                    nc.sync.dma_start(
                        out=h1T_sh[:, (mt + ti) * 128:(mt + ti + 1) * 128],
                        in_=hsb[:])
            else:
                nc.sync.dma_start(
                    out=bass.AP(out2[:, :].tensor, mt * 128 * OD,
                                [[OD, 128], [OD * 128, nb], [1, OD]]),
                    in_=vap(hs, 0, [[OD, nb], [1, OD]]))


_CACHE = {}


def kernel(**inputs):
    in_maps, meta = host_prep(**inputs)
    key = str(meta["T"])
    _CACHE["k"] = key
    if key not in _CACHE:
        nc = build_nc(meta)
        _CACHE[key] = (nc, make_runner(nc, NCORES))
    nc, run = _CACHE[key]
    results, best = run(in_maps, repeats=1)
    _CACHE["last_time"] = best
    out = np.empty((N, OUT_DIM), np.float32)
    for c in range(NCORES):
        out[c * NSH:(c + 1) * NSH] = results[c]["out2"][:NSH]
    return out



# revision 10
# speedup vs baseline: 1.3631x; 1.3631x over previous
"""GAT 2-layer kernel for TRN2, 8 NeuronCores (self-contained).

Strategy:
- dst-shard: core c owns nodes [c*12500, (c+1)*12500).
- Dense phases (x@W, h1@W2) replicated on all cores; folded weights give
  per-node [h | as | ad] in one matmul. Batched: 8 row-tiles per block,
  one load + one table write per block, PSUM groups of 4 (bank-aligned).
- Edge phase per core: 4 src-chunks (25000 nodes each, int16 gather reach),
  per-chunk degree-bucketed padded CSR over dst. dma_gather in up-to-32-col
  calls (4096 rows) pulls rows [h | as] (bf16, 512B L1 / 256B L2 stride).
  exp(leaky(as+ad)) and weighted feature sums on DVE/ACT, batched in
  24-col groups.
- Per-chunk partials [featsum | denom] -> DRAM staging (bf16), one write
  per bucket.
- Merge pass (natural node order, 16 tiles/batch): dma_gather the 4 partial
  rows per node, combine, per-head normalize, bias (+relu on L1).
- L1->L2: PE-transpose out1 -> h1T SBUF accum -> one DRAM write ->
  AllGather -> replicated dense2.
"""
import sys
sys.path.insert(0, "/opt/trn_rl_repo")
import numpy as np
import ml_dtypes

import concourse.bass as bass
import concourse.bacc as bacc
import concourse.tile as tile
from concourse import mybir
from concourse.library_config import mlp as mlp_lib


def make_runner(nc, n_cores):
    """PJRT runner: returns run_fn(in_maps, repeats) -> (results, best_time_s)."""
    import time
    import jax
    from jax.sharding import Mesh, PartitionSpec, NamedSharding
    from jax.experimental.shard_map import shard_map
    from concourse.bass2jax import (_bass_exec_p, install_neuronx_cc_hook,
                                    partition_id_tensor)
    install_neuronx_cc_hook()
    partition_name = nc.partition_id_tensor.name if nc.partition_id_tensor else None
    in_names, out_names, out_avals, zero_outs = [], [], [], []
    for alloc in nc.m.functions[0].allocations:
        if not isinstance(alloc, mybir.MemoryLocationSet):
            continue
        if not alloc.memorylocations:
            continue
        name = alloc.memorylocations[0].name
        if alloc.kind == "ExternalInput":
            if name != partition_name:
                in_names.append(name)
        elif alloc.kind == "ExternalOutput":
            out_names.append(name)
            shape = tuple(alloc.tensor_shape)
            dtype = mybir.dt.np(alloc.dtype)
            out_avals.append(jax.core.ShapedArray(shape, dtype))
            zero_outs.append(np.zeros(shape, dtype))
    n_params = len(in_names)
    n_outs = len(out_avals)
    all_in_names = list(in_names) + list(out_names)
    if partition_name is not None:
        all_in_names.append(partition_name)

    def _body(*args):
        operands = list(args)
        if partition_name is not None:
            operands.append(partition_id_tensor())
        return tuple(_bass_exec_p.bind(
            *operands, out_avals=tuple(out_avals), in_names=tuple(all_in_names),
            out_names=tuple(out_names), lowering_input_output_aliases=(),
            sim_require_finite=False, sim_require_nnan=False, nc=nc))

    devices = jax.devices()[:n_cores]
    mesh = Mesh(np.asarray(devices), ("core",))
    in_specs = (PartitionSpec("core"),) * (n_params + n_outs)
    out_specs = (PartitionSpec("core"),) * n_outs
    donate = tuple(range(n_params, n_params + n_outs))
    sharded = jax.jit(
        shard_map(_body, mesh=mesh, in_specs=in_specs, out_specs=out_specs,
                  check_rep=False),
        donate_argnums=donate, keep_unused=True)

    def run_fn(in_maps, repeats=1):
        per_core = [[np.asarray(m[name]) for name in in_names] for m in in_maps]
        concat_in = [np.concatenate([per_core[c][i] for c in range(n_cores)], 0)
                     for i in range(n_params)]
        sharding = NamedSharding(mesh, PartitionSpec("core"))
        dev_in = [jax.device_put(a, sharding) for a in concat_in]
        for a in dev_in:
            a.block_until_ready()
        times, out_arrs = [], None
        for _ in range(repeats):
            concat_zeros = [jax.device_put(
                np.zeros((n_cores * z.shape[0], *z.shape[1:]), z.dtype), sharding)
                for z in zero_outs]
            for z in concat_zeros:
                z.block_until_ready()
            t0 = time.perf_counter()
            out_arrs = sharded(*dev_in, *concat_zeros)
            for o in out_arrs:
                o.block_until_ready()
            times.append(time.perf_counter() - t0)
        results = [
            {name: np.asarray(out_arrs[i]).reshape(n_cores, *out_avals[i].shape)[c]
             for i, name in enumerate(out_names)}
            for c in range(n_cores)]
        return results, min(times)

    return run_fn

F32 = mybir.dt.float32
BF16 = mybir.dt.bfloat16
I16 = mybir.dt.int16
BF = ml_dtypes.bfloat16

NCORES = 8
N = 100000
IN_DIM = 128
HID = 32
OUT_DIM = 16
NSH = N // NCORES            # 12500
NT = 98                      # merge tiles per core
NSHP = NT * 128              # 12544
CH1 = 25000                  # table1 nodes per chunk
NCH = 4
CH1R = CH1 + 1               # +pad row
R2 = NCORES * NSHP           # 100352
CH2 = R2 // NCH              # 25088 (= 2 padded cores)
CH2R = CH2 + 1
BUCKETS = (1, 2, 3, 4, 6, 8, 12, 24)
NEG = -1.0e30
EPS = 1e-16
NQ = 4                       # SWDGE queues
MB = 16                      # merge batch (tiles)
GW = 8                       # gather cols per dma_gather call (ring: 1024 desc)
GC = 24                      # val/fs cols per DVE group (all BUCKETS divide)
XBF16 = True                 # x / W1 in bf16 for dense1

AluOp = mybir.AluOpType
ActFn = mybir.ActivationFunctionType
Axis = mybir.AxisListType


def _colgroups(D):
    out = []
    c = 0
    while c < D:
        w = min(8, D - c)
        out.append((c, w))
        c += w
    return out


def plan_segments(T):
    """Shared host/device plan (stream layout: slot (k,bi,t,d,p) lives at
    bucket_start + (t*D+d)*128 + p)."""
    calls = []
    rowbase = {}
    grid_rows = []
    off = 0
    for k in range(NCH):
        rb = 0
        for bi, D in enumerate(BUCKETS):
            for t in range(int(T[k][bi])):
                rowbase[(k, bi, t)] = rb
                for (c0, w) in _colgroups(D):
                    calls.append((k, bi, t, c0, w, off))
                    off += 128 * w
                rb += 128
        grid_rows.append(rb)
    return calls, rowbase, grid_rows, off


def _wrap_idx(flat):
    n = len(flat)
    assert n % 16 == 0
    w = np.asarray(flat, np.int16).reshape(n // 16, 16).T
    return np.ascontiguousarray(np.tile(w, (8, 1)))


def fold(W, a):
    Hh, F = a.shape
    w = np.zeros((W.shape[0], Hh), np.float32)
    for h in range(Hh):
        w[:, h] = W[:, h * F:(h + 1) * F] @ a[h]
    return w


def host_prep(x, edge_index, W1, a1_src, a1_dst, b1, W2, a2_src, a2_dst, b2):
    x = np.asarray(x, np.float32)
    ei = np.asarray(edge_index)
    src = ei[0].astype(np.int64)
    dst = ei[1].astype(np.int64)
    W1 = np.asarray(W1, np.float32)
    W2 = np.asarray(W2, np.float32)
    Waug1 = np.concatenate([W1, fold(W1, np.asarray(a1_src, np.float32)),
                            fold(W1, np.asarray(a1_dst, np.float32))], 1)
    Waug2 = np.concatenate([W2, fold(W2, np.asarray(a2_src, np.float32)),
                            fold(W2, np.asarray(a2_dst, np.float32))], 1)
    xT = np.ascontiguousarray(x.T)
    if XBF16:
        xT = xT.astype(BF)
        Waug1 = Waug1  # stays f32 host-side; device tile is bf16

    core_of = dst // NSH
    # ---- per-core, per-chunk CSR ----
    pc = []  # [core][chunk] = (deg, sorted_src_by_dst, starts)
    for c in range(NCORES):
        m = core_of == c
        s_c, d_c = src[m], dst[m] - c * NSH
        ch = s_c // CH1
        info = []
        for k in range(NCH):
            mk = ch == k
            sk, dk = s_c[mk], d_c[mk]
            deg = np.bincount(dk, minlength=NSH)
            order = np.argsort(dk, kind="stable")
            sk = sk[order]
            starts = np.zeros(NSH + 1, np.int64)
            np.cumsum(deg, out=starts[1:])
            info.append((deg, sk, starts))
        pc.append(info)

    # shared tile counts
    T = [[0] * len(BUCKETS) for _ in range(NCH)]
    for c in range(NCORES):
        for k in range(NCH):
            deg = pc[c][k][0]
            for bi, D in enumerate(BUCKETS):
                lo = BUCKETS[bi - 1] if bi else 0
                nb = int(((deg > lo) & (deg <= D)).sum())
                T[k][bi] = max(T[k][bi], (nb + 127) // 128)
            assert deg.max(initial=0) <= BUCKETS[-1], f"deg max {deg.max()}"
    calls, rowbase, grid_rows, stream_len = plan_segments(T)

    b1rep = np.tile(np.asarray(b1, np.float32)[None, :], (128, 1))
    b2rep = np.tile(np.asarray(b2, np.float32)[None, :], (128, 1))
    pad1 = np.zeros((1, 256), BF); pad1[0, 128:132] = NEG
    pad2 = np.zeros((1, 128), BF); pad2[0, 64:68] = NEG
    z256 = np.zeros((1, 256), BF)

    in_maps = []
    for c in range(NCORES):
        slot_nodes = []   # per chunk: grid row -> node (or -1)
        for k in range(NCH):
            gr = grid_rows[k]
            deg, sk, starts = pc[c][k]
            nodes_of = np.full(gr, -1, np.int64)
            for bi, D in enumerate(BUCKETS):
                lo = BUCKETS[bi - 1] if bi else 0
                nd = np.where((deg > lo) & (deg <= D))[0]
                rb = rowbase[(k, bi, 0)] if T[k][bi] else 0
                nodes_of[rb:rb + len(nd)] = nd
            slot_nodes.append(nodes_of)

        s1 = np.full(stream_len, CH1, np.int64)     # pad -> table1 chunk pad row
        s2 = np.full(stream_len, CH2, np.int64)     # pad -> table2 chunk pad row
        for (k, bi, t, c0, w, off) in calls:
            D = BUCKETS[bi]
            rb = rowbase[(k, bi, t)]
            deg, sk, starts = pc[c][k]
            nodes = slot_nodes[k][rb:rb + 128]
            j = off
            for d in range(c0, c0 + w):
                for p in range(128):
                    nd = nodes[p]
                    if nd >= 0 and d < starts[nd + 1] - starts[nd]:
                        s = sk[starts[nd] + d]
                        s1[j] = s % CH1
                        s2[j] = (s // NSH % 2) * NSHP + s % NSH
                    j += 1
        slot1w = _wrap_idx(s1)
        slot2w = _wrap_idx(s2)

        # ad idx: per (k, gridtile) 128 local dst ids (pad -> 0)
        adix = []
        for k in range(NCH):
            nd = slot_nodes[k]
            adix.append(np.where(nd >= 0, nd, 0))
        adw = _wrap_idx(np.concatenate(adix)) if stream_len else None

        # merge idx: per chunk, per natural node (padded to NSHP): grid row or zero-row
        mrg = []
        for k in range(NCH):
            deg = pc[c][k][0]
            pos = np.full(NSHP, grid_rows[k], np.int64)  # zero row
            nd = slot_nodes[k]
            real = nd >= 0
            pos[nd[real]] = np.nonzero(real)[0]
            mrg.append(pos)
        mrgw = _wrap_idx(np.concatenate(mrg))

        in_maps.append(dict(
            xT=xT, Waug1=Waug1, Waug2=Waug2.astype(BF),
            b1rep=b1rep, b2rep=b2rep, pad1=pad1, pad2=pad2, z256=z256,
            slot1w=slot1w, slot2w=slot2w, adw=adw, mrgw=mrgw,
        ))
    meta = dict(T=T, calls=calls, rowbase=rowbase, grid_rows=grid_rows,
                stream_len=stream_len)
    return in_maps, meta


def vap(t, off, dims):
    a = t[:]
    return bass.AP(a.tensor, a.offset + off, [list(a.ap[0])] + [list(d) for d in dims])


def build_nc(meta):
    T = meta["T"]
    grid_rows = meta["grid_rows"]
    stream_len = meta["stream_len"]
    SW = stream_len // 16
    ADL = sum(grid_rows)
    AW = ADL // 16
    MW = (NCH * NSHP) // 16
    XDT = BF16 if XBF16 else F32

    nc = bacc.Bacc("TRN2", target_bir_lowering=False, num_swdge_queues=NQ)
    dp = nc.declare_dram_parameter
    xT = dp("xT", [IN_DIM, N], XDT, isOutput=False)
    Waug1 = dp("Waug1", [128, 136], F32, isOutput=False)
    Waug2 = dp("Waug2", [32, 72], BF16, isOutput=False)
    b1rep = dp("b1rep", [128, HID], F32, isOutput=False)
    b2rep = dp("b2rep", [128, OUT_DIM], F32, isOutput=False)
    pad1 = dp("pad1", [1, 256], BF16, isOutput=False)
    pad2 = dp("pad2", [1, 128], BF16, isOutput=False)
    z256 = dp("z256", [1, 256], BF16, isOutput=False)
    slot1w = dp("slot1w", [128, SW], I16, isOutput=False)
    slot2w = dp("slot2w", [128, SW], I16, isOutput=False)
    adw = dp("adw", [128, AW], I16, isOutput=False)
    mrgw = dp("mrgw", [128, MW], I16, isOutput=False)
    out2 = dp("out2", [NSHP, OUT_DIM], F32, isOutput=True)

    table1 = nc.dram_tensor("table1", [NCH * CH1R, 256], BF16)
    table2 = nc.dram_tensor("table2", [NCH * CH2R, 128], BF16)
    ad1nat = nc.dram_tensor("ad1nat", [N, 4], F32)
    ad2nat = nc.dram_tensor("ad2nat", [R2, 4], F32)
    ad1c = nc.dram_tensor("ad1c", [NSHP, 64], F32)
    ad2c = nc.dram_tensor("ad2c", [NSHP, 64], F32)
    stg1 = [nc.dram_tensor(f"stg1_{k}", [grid_rows[k] + 1, 256], BF16)
            for k in range(NCH)]
    stg2 = [nc.dram_tensor(f"stg2_{k}", [grid_rows[k] + 1, 128], BF16)
            for k in range(NCH)]
    h1T_sh = nc.dram_tensor("h1T_sh", [32, NSHP], BF16)
    h1T_all = nc.dram_tensor("h1T_all", [NCORES, 32, NSHP], BF16,
                             addr_space="Shared")

    qn = [0]
    def nextq():
        qn[0] = (qn[0] + 1) % NQ
        return qn[0]

    with tile.TileContext(nc) as tc:
        nc.gpsimd.load_library(mlp_lib)

        # ---------- consts / pads ----------
        with tc.tile_pool(name="konst", bufs=1) as kp:
            w1sb = kp.tile([128, 136], XDT)
            if XBF16:
                w1f = kp.tile([128, 136], F32)
                nc.sync.dma_start(out=w1f[:], in_=Waug1[:, :])
                nc.vector.tensor_copy(out=w1sb[:], in_=w1f[:])
            else:
                nc.sync.dma_start(out=w1sb[:], in_=Waug1[:, :])
            w2sb = kp.tile([32, 72], BF16)
            nc.sync.dma_start(out=w2sb[:], in_=Waug2[:, :])
            b1sb = kp.tile([128, HID], F32)
            nc.sync.dma_start(out=b1sb[:], in_=b1rep[:, :])
            b2sb = kp.tile([128, OUT_DIM], F32)
            nc.sync.dma_start(out=b2sb[:], in_=b2rep[:, :])
            for k in range(NCH):
                nc.sync.dma_start(out=table1[k * CH1R + CH1, :], in_=pad1[0, :])
                nc.sync.dma_start(out=table2[k * CH2R + CH2, :], in_=pad2[0, :])
                nc.sync.dma_start(out=stg1[k][grid_rows[k], :], in_=z256[0, :])
                nc.sync.dma_start(out=stg2[k][grid_rows[k], :], in_=z256[0, :128])

            # ---------- dense1 ----------
            with nc.named_scope("dense1"):
                _dense1(nc, tc, xT, w1sb, table1, ad1nat, XDT)

            # ---------- repack ad1: own 12500 rows -> ad1c [NSHP, 64] ----------
            pid = nc.gpsimd.partition_id()
            with nc.named_scope("repack1"), tc.tile_pool(name="rp", bufs=2) as rp:
                adt = rp.tile([128, 98 * 4], F32, tag="adt")
                nc.gpsimd.memset(adt[:], 0.0)
                base = pid * NSH
                nc.gpsimd.dma_start(
                    out=vap(adt, 0, [[4, 97], [1, 4]]),
                    in_=ad1nat[bass.ds(base, 12416), :]
                        .rearrange("(t p) f -> p t f", p=128))
                nc.gpsimd.dma_start(
                    out=bass.AP(adt[:].tensor, adt[:].offset + 97 * 4,
                                [[list(adt[:].ap[0])[0], 84], [1, 4]]),
                    in_=ad1nat[bass.ds(base + 12416, 84), :])
                nc.sync.dma_start(
                    out=bass.AP(ad1c[:, :].tensor, 0,
                                [[64, 128], [64 * 128, 98], [1, 4]]),
                    in_=adt[:])

            # ---------- edge pass L1 ----------
            with nc.named_scope("edge1"):
                _edge_pass(nc, tc, meta, layer=1, slotw=slot1w, adw=adw,
                           table=table1, stg=stg1, ad_core=ad1c, nextq=nextq)

            # ---------- merge L1 -> h1T ----------
            with nc.named_scope("merge1"):
                _merge_pass(nc, tc, meta, layer=1, mrgw=mrgw, stg=stg1,
                            bsb=b1sb, out2=None, h1T_sh=h1T_sh, nextq=nextq)

            # ---------- allgather ----------
            with nc.named_scope("allgather"):
                nc.gpsimd.collective_compute(
                    "AllGather", AluOp.bypass,
                    replica_groups=[list(range(NCORES))],
                    ins=[h1T_sh[:, :]], outs=[h1T_all[:, :, :]])

            # ---------- dense2 ----------
            with nc.named_scope("dense2"):
                _dense2(nc, tc, h1T_all, w2sb, table2, ad2nat)

            # ---------- repack ad2 ----------
            with nc.named_scope("repack2"), tc.tile_pool(name="rp2", bufs=2) as rp:
                adt = rp.tile([128, 98 * 4], F32, tag="adt2")
                base2 = pid * NSHP
                nc.gpsimd.dma_start(
                    out=vap(adt, 0, [[4, 98], [1, 4]]),
                    in_=ad2nat[bass.ds(base2, NSHP), :]
                        .rearrange("(t p) f -> p t f", p=128))
                nc.sync.dma_start(
                    out=bass.AP(ad2c[:, :].tensor, 0,
                                [[64, 128], [64 * 128, 98], [1, 4]]),
                    in_=adt[:])

            # ---------- edge pass L2 ----------
            with nc.named_scope("edge2"):
                _edge_pass(nc, tc, meta, layer=2, slotw=slot2w, adw=adw,
                           table=table2, stg=stg2, ad_core=ad2c, nextq=nextq)

            # ---------- merge L2 -> out2 ----------
            with nc.named_scope("merge2"):
                _merge_pass(nc, tc, meta, layer=2, mrgw=mrgw, stg=stg2,
                            bsb=b2sb, out2=out2, h1T_sh=None, nextq=nextq)

    nc.finalize()
    return nc


def _row_splits(n0, nn, chunk, chunkr):
    """Split node range [n0, n0+nn) at `chunk` boundaries; yield
    (node_start, count, table_row_start)."""
    a = n0
    while a < n0 + nn:
        k = a // chunk
        b = min(n0 + nn, (k + 1) * chunk)
        yield a, b - a, k * chunkr + (a - k * chunk)
        a = b


def _write_rows(nc, dst_t, n0, nn, sb, slot, width, rowstride, chunk, chunkr):
    """Write nn rows (nodes n0..) from SBUF tile sb [128, BB*slot] where
    node n0+b*128+p lives at partition p, free offset b*slot, width elems.
    dst row stride `rowstride` elems, chunk-shifted table rows."""
    for (a, cnt, r0) in _row_splits(n0, nn, chunk, chunkr):
        off = a - n0                       # row offset within block
        while cnt > 0:
            b, p = off // 128, off % 128
            if p == 0 and cnt >= 128:
                nbt = cnt // 128
                nc.sync.dma_start(
                    out=bass.AP(dst_t[:, :].tensor, r0 * rowstride,
                                [[rowstride, 128], [rowstride * 128, nbt],
                                 [1, width]]),
                    in_=vap(sb, b * slot, [[slot, nbt], [1, width]]))
                took = nbt * 128
            else:
                take = min(cnt, 128 - p)
                ap = sb[p:p + take]
                nc.sync.dma_start(
                    out=bass.AP(dst_t[:, :].tensor, r0 * rowstride,
                                [[rowstride, take], [1, width]]),
                    in_=bass.AP(ap.tensor, ap.offset + b * slot,
                                [list(ap.ap[0]), [1, width]]))
                took = take
            r0 += took
            off += took
            cnt -= took


BB = 8  # row-tiles per dense block


def _dense1(nc, tc, xT, w1sb, table1, ad1nat, XDT):
    with (tc.tile_pool(name="d1", bufs=3) as dpool,
          tc.tile_pool(name="d1p", bufs=2, space="PSUM") as dps):
        n0 = 0
        while n0 < N:
            nn = min(BB * 128, N - n0)
            bt = (nn + 127) // 128
            xm8 = dpool.tile([128, BB * 128], XDT, tag="xm8")
            nc.sync.dma_start(out=xm8[:, 0:nn], in_=bass.AP(
                xT[:, :].tensor, n0, [[N, 128], [1, nn]]))
            hrow8 = dpool.tile([128, BB * 132], BF16, tag="hrow8")
            adsb8 = dpool.tile([128, BB * 4], F32, tag="adsb8")
            for g0 in range(0, bt, 4):
                ng = min(4, bt - g0)
                ps = dps.tile([128, 2048], F32, tag="ps")
                for b in range(ng):
                    tb = g0 + b
                    nnb = min(128, nn - tb * 128)
                    nc.tensor.matmul(
                        out=ps[0:nnb, b * 512:b * 512 + 136],
                        lhsT=xm8[:, tb * 128:tb * 128 + nnb],
                        rhs=w1sb[:], start=True, stop=True)
                nc.vector.tensor_copy(
                    out=vap(hrow8, g0 * 132, [[132, ng], [1, 132]]),
                    in_=vap(ps, 0, [[512, ng], [1, 132]]))
                nc.vector.tensor_copy(
                    out=vap(adsb8, g0 * 4, [[4, ng], [1, 4]]),
                    in_=vap(ps, 132, [[512, ng], [1, 4]]))
            _write_rows(nc, table1, n0, nn, hrow8, 132, 132, 256, CH1, CH1R)
            _write_rows(nc, ad1nat, n0, nn, adsb8, 4, 4, 4, N, N)
            n0 += nn


def _dense2(nc, tc, h1T_all, w2sb, table2, ad2nat):
    with (tc.tile_pool(name="d2", bufs=3) as dpool,
          tc.tile_pool(name="d2p", bufs=2, space="PSUM") as dps):
        for cc in range(NCORES):
            shift = cc // 2                # table2 pad-row shift, const per cc
            q0 = 0
            while q0 < NT:
                nb = min(BB, NT - q0)
                lh8 = dpool.tile([32, BB * 128], BF16, tag="lh8")
                nc.sync.dma_start(out=lh8[:, 0:nb * 128], in_=bass.AP(
                    h1T_all[:, :, :].tensor, cc * 32 * NSHP + q0 * 128,
                    [[NSHP, 32], [1, nb * 128]]))
                h2row8 = dpool.tile([128, BB * 68], BF16, tag="h2row8")
                adsb8 = dpool.tile([128, BB * 4], F32, tag="adsb28")
                for g0 in range(0, nb, 4):
                    ng = min(4, nb - g0)
                    ps = dps.tile([128, 2048], F32, tag="ps2")
                    for b in range(ng):
                        nc.tensor.matmul(
                            out=ps[:, b * 512:b * 512 + 72],
                            lhsT=lh8[:, (g0 + b) * 128:(g0 + b + 1) * 128],
                            rhs=w2sb[:], start=True, stop=True)
                    nc.vector.tensor_copy(
                        out=vap(h2row8, g0 * 68, [[68, ng], [1, 68]]),
                        in_=vap(ps, 0, [[512, ng], [1, 68]]))
                    nc.vector.tensor_copy(
                        out=vap(adsb8, g0 * 4, [[4, ng], [1, 4]]),
                        in_=vap(ps, 68, [[512, ng], [1, 4]]))
                r0 = (cc * NT + q0) * 128 + shift
                nc.sync.dma_start(
                    out=bass.AP(table2[:, :].tensor, r0 * 128,
                                [[128, 128], [128 * 128, nb], [1, 68]]),
                    in_=vap(h2row8, 0, [[68, nb], [1, 68]]))
                a0 = (cc * NT + q0) * 128
                nc.sync.dma_start(
                    out=bass.AP(ad2nat[:, :].tensor, a0 * 4,
                                [[4, 128], [4 * 128, nb], [1, 4]]),
                    in_=vap(adsb8, 0, [[4, nb], [1, 4]]))
                q0 += nb


def _edge_pass(nc, tc, meta, layer, slotw, adw, table, stg, ad_core, nextq):
    rowbase = meta["rowbase"]
    grid_rows = meta["grid_rows"]
    RW = 256 if layer == 1 else 128       # table row elems (bf16)
    FD = 128 if layer == 1 else 64        # feature elems
    SW = meta["stream_len"] // 16
    AW = sum(grid_rows) // 16
    KR = CH1R if layer == 1 else CH2R

    # stream offset of each bucket's first slot (buckets are contiguous)
    bstart = {}
    off = 0
    for k in range(NCH):
        for bi, D in enumerate(BUCKETS):
            bstart[(k, bi)] = off
            off += int(meta["T"][k][bi]) * D * 128

    with (tc.tile_pool(name=f"eidx{layer}", bufs=1) as ip,
          tc.tile_pool(name=f"eg{layer}", bufs=2) as gp,
          tc.tile_pool(name=f"ea{layer}", bufs=2) as ap_pool,
          tc.tile_pool(name=f"ew{layer}", bufs=2) as wp):
        sidx = ip.tile([128, SW], I16, tag="sidx")
        nc.sync.dma_start(out=sidx[:], in_=slotw[:, :])
        aidx = ip.tile([128, AW], I16, tag="aidx")
        nc.sync.dma_start(out=aidx[:], in_=adw[:, :])

        abase = 0
        for k in range(NCH):
            for bi, D in enumerate(BUCKETS):
                Tb = int(meta["T"][k][bi])
                if Tb == 0:
                    continue
                ncols = Tb * D
                # ad gather for this bucket's grid tiles
                ADG = ap_pool.tile([128, Tb, 64], F32, tag="ADG")
                na = Tb * 128
                o = 0
                while o < na:
                    nb = min(1024, na - o)
                    nc.gpsimd.dma_gather(
                        ADG[:, (o // 128):(o + nb) // 128, :], ad_core[:, :],
                        aidx[:, (abase + o) // 16:(abase + o + nb) // 16],
                        nb, nb, 64, queue_num=nextq())
                    o += nb
                abase += na
                # slot gather
                G = gp.tile([128, ncols, RW], BF16, tag="G")
                off = bstart[(k, bi)]
                c = 0
                while c < ncols:
                    w = min(GW, ncols - c)
                    nc.gpsimd.dma_gather(
                        G[:, c:c + w, :], table[k * KR:(k + 1) * KR, :],
                        sidx[:, (off + 128 * c) // 16:(off + 128 * (c + w)) // 16],
                        128 * w, 128 * w, RW, queue_num=nextq())
                    c += w
                # e = exp(leaky(as + ad))
                e = wp.tile([128, ncols * 4], F32, tag="e")
                nc.vector.tensor_tensor(
                    out=e[:], in0=vap(G, FD, [[RW, ncols], [1, 4]]),
                    in1=vap(ADG, 0, [[64, Tb], [0, D], [1, 4]]),
                    op=AluOp.add)
                nc.vector.scalar_tensor_tensor(
                    out=e[:], in0=e[:], scalar=0.2, in1=e[:],
                    op0=AluOp.mult, op1=AluOp.max)
                nc.scalar.activation(out=e[:], in_=e[:], func=ActFn.Exp)
                den = wp.tile([128, Tb * 4], F32, tag="den")
                nc.vector.tensor_reduce(
                    out=den[:, 0:Tb * 4],
                    in_=vap(e, 0, [[4 * D, Tb], [1, 4], [4, D]]),
                    axis=Axis.X, op=AluOp.add)
                # weighted feature sums, GC cols per group
                fs_all = wp.tile([128, Tb * FD], F32, tag="fs")
                so_all = wp.tile([128, Tb, RW], BF16, tag="so")
                for c0 in range(0, ncols, GC):
                    gc = min(GC, ncols - c0)
                    tg = gc // D
                    t0 = c0 // D
                    val = wp.tile([128, GC * FD], F32, tag="val")
                    nc.vector.tensor_tensor(
                        out=val[:, 0:gc * FD],
                        in0=vap(G, c0 * RW, [[RW, gc], [FD // 4, 4], [1, FD // 4]]),
                        in1=vap(e, c0 * 4, [[4, gc], [1, 4], [0, FD // 4]]),
                        op=AluOp.mult)
                    nc.vector.tensor_reduce(
                        out=vap(fs_all, t0 * FD, [[FD, tg], [1, FD]]),
                        in_=vap(val, 0, [[D * FD, tg], [1, FD], [FD, D]]),
                        axis=Axis.X, op=AluOp.add)
                    nc.vector.tensor_copy(
                        out=vap(so_all, t0 * RW, [[RW, tg], [1, FD]]),
                        in_=vap(fs_all, t0 * FD, [[FD, tg], [1, FD]]))
                nc.vector.tensor_copy(
                    out=vap(so_all, FD, [[RW, Tb], [1, 4]]),
                    in_=vap(den, 0, [[4, Tb], [1, 4]]))
                rb = rowbase[(k, bi, 0)]
                nc.sync.dma_start(
                    out=bass.AP(stg[k][:, :].tensor, rb * RW,
                                [[RW, 128], [RW * 128, Tb], [1, FD + 4]]),
                    in_=vap(so_all, 0, [[RW, Tb], [1, FD + 4]]))


def _merge_pass(nc, tc, meta, layer, mrgw, stg, bsb, out2, h1T_sh, nextq):
    grid_rows = meta["grid_rows"]
    RW = 256 if layer == 1 else 128
    FD = 128 if layer == 1 else 64
    OD = HID if layer == 1 else OUT_DIM
    MW = (NCH * NSHP) // 16
    W = FD + 4

    with (tc.tile_pool(name=f"midx{layer}", bufs=1) as ip,
          tc.tile_pool(name=f"mg{layer}", bufs=2) as gp,
          tc.tile_pool(name=f"mw{layer}", bufs=2) as wp,
          tc.tile_pool(name=f"mp{layer}", bufs=2, space="PSUM") as pp):
        midx = ip.tile([128, MW], I16, tag="midx")
        nc.sync.dma_start(out=midx[:], in_=mrgw[:, :])
        if layer == 1:
            from concourse.masks import make_identity
            ident = ip.tile([128, 128], F32, tag="ident")
            make_identity(nc, ident[:])
            h1sb = ip.tile([32, NSHP], BF16, tag="h1sb")

        mt = 0
        while mt < NT:
            nb = min(MB, NT - mt)
            Gs = []
            for k in range(NCH):
                Gk = gp.tile([128, MB, RW], BF16, tag=f"MG{k}")
                for b0 in range(0, nb, 8):
                    bn = min(8, nb - b0)
                    ioff = k * NSHP + (mt + b0) * 128
                    nc.gpsimd.dma_gather(
                        Gk[:, b0:b0 + bn, :], stg[k][:, :],
                        midx[:, ioff // 16:(ioff + bn * 128) // 16],
                        bn * 128, bn * 128, RW, queue_num=nextq())
                Gs.append(Gk)
            s01 = wp.tile([128, MB * W], F32, tag="s01")
            nc.vector.tensor_tensor(
                out=vap(s01, 0, [[W, nb], [1, W]]),
                in0=vap(Gs[0], 0, [[RW, nb], [1, W]]),
                in1=vap(Gs[1], 0, [[RW, nb], [1, W]]), op=AluOp.add)
            s23 = wp.tile([128, MB * W], F32, tag="s23")
            nc.vector.tensor_tensor(
                out=vap(s23, 0, [[W, nb], [1, W]]),
                in0=vap(Gs[2], 0, [[RW, nb], [1, W]]),
                in1=vap(Gs[3], 0, [[RW, nb], [1, W]]), op=AluOp.add)
            nc.vector.tensor_tensor(
                out=vap(s01, 0, [[W, nb], [1, W]]),
                in0=vap(s01, 0, [[W, nb], [1, W]]),
                in1=vap(s23, 0, [[W, nb], [1, W]]), op=AluOp.add)
            rec = wp.tile([128, MB * 4], F32, tag="rec")
            nc.vector.tensor_scalar_add(
                out=vap(rec, 0, [[4, nb], [1, 4]]),
                in0=vap(s01, FD, [[W, nb], [1, 4]]), scalar1=EPS)
            nc.vector.reciprocal(out=rec[:, 0:nb * 4], in_=rec[:, 0:nb * 4])
            nc.vector.tensor_scalar_mul(out=rec[:, 0:nb * 4],
                                        in0=rec[:, 0:nb * 4], scalar1=0.25)
            sc = wp.tile([128, MB * FD], F32, tag="sc")
            nc.vector.tensor_tensor(
                out=vap(sc, 0, [[FD, nb], [FD // 4, 4], [1, FD // 4]]),
                in0=vap(s01, 0, [[W, nb], [FD // 4, 4], [1, FD // 4]]),
                in1=vap(rec, 0, [[4, nb], [1, 4], [0, FD // 4]]),
                op=AluOp.mult)
            hs = wp.tile([128, MB * OD], F32, tag="hs")
            nc.vector.tensor_reduce(
                out=vap(hs, 0, [[OD, nb], [1, OD]]),
                in_=vap(sc, 0, [[FD, nb], [1, OD], [OD, 4]]),
                axis=Axis.X, op=AluOp.add)
            nc.vector.tensor_tensor(
                out=vap(hs, 0, [[OD, nb], [1, OD]]),
                in0=vap(hs, 0, [[OD, nb], [1, OD]]),
                in1=vap(bsb, 0, [[0, nb], [1, OD]]), op=AluOp.add)
            if layer == 1:
                nc.scalar.activation(out=hs[:, 0:nb * OD], in_=hs[:, 0:nb * OD],
                                     func=ActFn.Relu)
                for g0 in range(0, nb, 4):
                    gn = min(4, nb - g0)
                    psT = pp.tile([32, 512], F32, tag="psT")
                    for j in range(gn):
                        nc.tensor.transpose(
                            out=psT[:, j * 128:(j + 1) * 128],
                            in_=hs[:, (g0 + j) * OD:(g0 + j + 1) * OD],
                            identity=ident[:])
                    nc.vector.tensor_copy(
                        out=h1sb[:, (mt + g0) * 128:(mt + g0 + gn) * 128],
                        in_=psT[:, 0:gn * 128])
            else:
                nc.sync.dma_start(
                    out=bass.AP(out2[:, :].tensor, mt * 128 * OD,
                                [[OD, 128], [OD * 128, nb], [1, OD]]),
                    in_=vap(hs, 0, [[OD, nb], [1, OD]]))
            mt += nb
        if layer == 1:
            nc.sync.dma_start(out=h1T_sh[:, :], in_=h1sb[:])


_CACHE = {}


def kernel(**inputs):
    in_maps, meta = host_prep(**inputs)
    key = str(meta["T"])
    _CACHE["k"] = key
    if key not in _CACHE:
        nc = build_nc(meta)
        _CACHE[key] = (nc, make_runner(nc, NCORES))
    nc, run = _CACHE[key]
    results, best = run(in_maps, repeats=1)
    _CACHE["last_time"] = best
    out = np.empty((N, OUT_DIM), np.float32)
    for c in range(NCORES):
        out[c * NSH:(c + 1) * NSH] = results[c]["out2"][:NSH]
    return out


# revision 18
# speedup vs baseline: 2.9859x; 2.1905x over previous
"""GAT 2-layer kernel for TRN2, 8 NeuronCores (self-contained).

Strategy:
- dst-shard: core c owns nodes [c*12500, (c+1)*12500).
- Dense phases (x@W, h1@W2) replicated on all cores; folded weights give
  per-node [h | as | ad] in one matmul. Batched: 8 row-tiles per block,
  one load + one table write per block, PSUM groups of 4 (bank-aligned).
- Edge phase per core: 4 src-chunks (25000 nodes each, int16 gather reach),
  per-chunk degree-bucketed padded CSR over dst. dma_gather in up-to-32-col
  calls (4096 rows) pulls rows [h | as] (bf16, 512B L1 / 256B L2 stride).
  exp(leaky(as+ad)) and weighted feature sums on DVE/ACT, batched in
  24-col groups.
- Per-chunk partials [featsum | denom] -> DRAM staging (bf16), one write
  per bucket.
- Merge pass (natural node order, 16 tiles/batch): dma_gather the 4 partial
  rows per node, combine, per-head normalize, bias (+relu on L1).
- L1->L2: PE-transpose out1 -> h1T SBUF accum -> one DRAM write ->
  AllGather -> replicated dense2.
"""
import sys
sys.path.insert(0, "/opt/trn_rl_repo")
import numpy as np
import ml_dtypes

import concourse.bass as bass
import concourse.bacc as bacc
import concourse.tile as tile
from concourse import mybir
from concourse.library_config import mlp as mlp_lib


def make_runner(nc, n_cores):
    """PJRT runner: returns run_fn(in_maps, repeats) -> (results, best_time_s)."""
    import time
    import jax
    from jax.sharding import Mesh, PartitionSpec, NamedSharding
    from jax.experimental.shard_map import shard_map
    from concourse.bass2jax import (_bass_exec_p, install_neuronx_cc_hook,
                                    partition_id_tensor)
    install_neuronx_cc_hook()
    partition_name = nc.partition_id_tensor.name if nc.partition_id_tensor else None
    in_names, out_names, out_avals, zero_outs = [], [], [], []
    for alloc in nc.m.functions[0].allocations:
        if not isinstance(alloc, mybir.MemoryLocationSet):
            continue
        if not alloc.memorylocations:
            continue
        name = alloc.memorylocations[0].name
        if alloc.kind == "ExternalInput":
            if name != partition_name:
                in_names.append(name)
        elif alloc.kind == "ExternalOutput":
            out_names.append(name)
            shape = tuple(alloc.tensor_shape)
            dtype = mybir.dt.np(alloc.dtype)
            out_avals.append(jax.core.ShapedArray(shape, dtype))
            zero_outs.append(np.zeros(shape, dtype))
    n_params = len(in_names)
    n_outs = len(out_avals)
    all_in_names = list(in_names) + list(out_names)
    if partition_name is not None:
        all_in_names.append(partition_name)

    def _body(*args):
        operands = list(args)
        if partition_name is not None:
            operands.append(partition_id_tensor())
        return tuple(_bass_exec_p.bind(
            *operands, out_avals=tuple(out_avals), in_names=tuple(all_in_names),
            out_names=tuple(out_names), lowering_input_output_aliases=(),
            sim_require_finite=False, sim_require_nnan=False, nc=nc))

    devices = jax.devices()[:n_cores]
    mesh = Mesh(np.asarray(devices), ("core",))
    in_specs = (PartitionSpec("core"),) * (n_params + n_outs)
    out_specs = (PartitionSpec("core"),) * n_outs
    donate = tuple(range(n_params, n_params + n_outs))
    sharded = jax.jit(
        shard_map(_body, mesh=mesh, in_specs=in_specs, out_specs=out_specs,
                  check_rep=False),
        donate_argnums=donate, keep_unused=True)

    def run_fn(in_maps, repeats=1):
        per_core = [[np.asarray(m[name]) for name in in_names] for m in in_maps]
        concat_in = [np.concatenate([per_core[c][i] for c in range(n_cores)], 0)
                     for i in range(n_params)]
        sharding = NamedSharding(mesh, PartitionSpec("core"))
        dev_in = [jax.device_put(a, sharding) for a in concat_in]
        for a in dev_in:
            a.block_until_ready()
        times, out_arrs = [], None
        for _ in range(repeats):
            concat_zeros = [jax.device_put(
                np.zeros((n_cores * z.shape[0], *z.shape[1:]), z.dtype), sharding)
                for z in zero_outs]
            for z in concat_zeros:
                z.block_until_ready()
            t0 = time.perf_counter()
            out_arrs = sharded(*dev_in, *concat_zeros)
            for o in out_arrs:
                o.block_until_ready()
            times.append(time.perf_counter() - t0)
        results = [
            {name: np.asarray(out_arrs[i]).reshape(n_cores, *out_avals[i].shape)[c]
             for i, name in enumerate(out_names)}
            for c in range(n_cores)]
        return results, min(times)

    return run_fn

F32 = mybir.dt.float32
BF16 = mybir.dt.bfloat16
I16 = mybir.dt.int16
BF = ml_dtypes.bfloat16

NCORES = 8
N = 100000
IN_DIM = 128
HID = 32
OUT_DIM = 16
NSH = N // NCORES            # 12500
NT = 98                      # merge tiles per core
NSHP = NT * 128              # 12544
CH1 = 25000                  # table1 nodes per chunk
NCH = 4
CH1R = CH1 + 1               # +pad row
R2 = NCORES * NSHP           # 100352
CH2 = R2 // NCH              # 25088 (= 2 padded cores)
CH2R = CH2 + 1
BUCKETS = (1, 2, 3, 4, 6, 8, 12, 24)
NEG = -1.0e30
EPS = 1e-16
NQ = 4                       # SWDGE queues
MB = 16                      # merge batch (tiles)
SCRATCH = 16384              # SWDGE ring: SCRATCH//16 descriptors per queue
MAXG = SCRATCH // 16         # max rows per dma_gather call
GW = MAXG // 128             # gather cols per dma_gather call
GC = 24                      # val/fs cols per DVE group (all BUCKETS divide)
XBF16 = True                 # x / W1 in bf16 for dense1

AluOp = mybir.AluOpType
ActFn = mybir.ActivationFunctionType
Axis = mybir.AxisListType


def _colgroups(D):
    out = []
    c = 0
    while c < D:
        w = min(8, D - c)
        out.append((c, w))
        c += w
    return out


def plan_segments(T):
    """Shared host/device plan (stream layout: slot (k,bi,t,d,p) lives at
    bucket_start + (t*D+d)*128 + p)."""
    calls = []
    rowbase = {}
    grid_rows = []
    off = 0
    for k in range(NCH):
        rb = 0
        for bi, D in enumerate(BUCKETS):
            for t in range(int(T[k][bi])):
                rowbase[(k, bi, t)] = rb
                for (c0, w) in _colgroups(D):
                    calls.append((k, bi, t, c0, w, off))
                    off += 128 * w
                rb += 128
        grid_rows.append(rb)
    return calls, rowbase, grid_rows, off


def _wrap_idx(flat):
    n = len(flat)
    assert n % 16 == 0
    w = np.asarray(flat, np.int16).reshape(n // 16, 16).T
    return np.ascontiguousarray(np.tile(w, (8, 1)))


def fold(W, a):
    Hh, F = a.shape
    w = np.zeros((W.shape[0], Hh), np.float32)
    for h in range(Hh):
        w[:, h] = W[:, h * F:(h + 1) * F] @ a[h]
    return w


def host_prep(x, edge_index, W1, a1_src, a1_dst, b1, W2, a2_src, a2_dst, b2):
    x = np.asarray(x, np.float32)
    ei = np.asarray(edge_index)
    src = ei[0].astype(np.int64)
    dst = ei[1].astype(np.int64)
    W1 = np.asarray(W1, np.float32)
    W2 = np.asarray(W2, np.float32)
    Waug1 = np.concatenate([W1, fold(W1, np.asarray(a1_src, np.float32)),
                            fold(W1, np.asarray(a1_dst, np.float32))], 1)
    Waug2 = np.concatenate([W2, fold(W2, np.asarray(a2_src, np.float32)),
                            fold(W2, np.asarray(a2_dst, np.float32))], 1)
    xT = np.ascontiguousarray(x.T)
    if XBF16:
        xT = xT.astype(BF)
        Waug1 = Waug1  # stays f32 host-side; device tile is bf16

    core_of = dst // NSH
    # ---- per-core, per-chunk CSR ----
    pc = []  # [core][chunk] = (deg, sorted_src_by_dst, starts)
    for c in range(NCORES):
        m = core_of == c
        s_c, d_c = src[m], dst[m] - c * NSH
        ch = s_c // CH1
        info = []
        for k in range(NCH):
            mk = ch == k
            sk, dk = s_c[mk], d_c[mk]
            deg = np.bincount(dk, minlength=NSH)
            order = np.argsort(dk, kind="stable")
            sk = sk[order]
            starts = np.zeros(NSH + 1, np.int64)
            np.cumsum(deg, out=starts[1:])
            info.append((deg, sk, starts))
        pc.append(info)

    # shared tile counts
    T = [[0] * len(BUCKETS) for _ in range(NCH)]
    for c in range(NCORES):
        for k in range(NCH):
            deg = pc[c][k][0]
            for bi, D in enumerate(BUCKETS):
                lo = BUCKETS[bi - 1] if bi else 0
                nb = int(((deg > lo) & (deg <= D)).sum())
                T[k][bi] = max(T[k][bi], (nb + 127) // 128)
            assert deg.max(initial=0) <= BUCKETS[-1], f"deg max {deg.max()}"
    calls, rowbase, grid_rows, stream_len = plan_segments(T)

    b1rep = np.tile(np.asarray(b1, np.float32)[None, :], (128, 1))
    b2rep = np.tile(np.asarray(b2, np.float32)[None, :], (128, 1))
    pad1 = np.zeros((1, 256), BF); pad1[0, 128:132] = NEG
    pad2 = np.zeros((1, 128), BF); pad2[0, 64:68] = NEG
    z256 = np.zeros((1, 256), BF)

    in_maps = []
    for c in range(NCORES):
        slot_nodes = []   # per chunk: grid row -> node (or -1)
        for k in range(NCH):
            gr = grid_rows[k]
            deg, sk, starts = pc[c][k]
            nodes_of = np.full(gr, -1, np.int64)
            for bi, D in enumerate(BUCKETS):
                lo = BUCKETS[bi - 1] if bi else 0
                nd = np.where((deg > lo) & (deg <= D))[0]
                rb = rowbase[(k, bi, 0)] if T[k][bi] else 0
                nodes_of[rb:rb + len(nd)] = nd
            slot_nodes.append(nodes_of)

        s1 = np.full(stream_len, CH1, np.int64)     # pad -> table1 chunk pad row
        s2 = np.full(stream_len, CH2, np.int64)     # pad -> table2 chunk pad row
        for (k, bi, t, c0, w, off) in calls:
            D = BUCKETS[bi]
            rb = rowbase[(k, bi, t)]
            deg, sk, starts = pc[c][k]
            nodes = slot_nodes[k][rb:rb + 128]
            j = off
            for d in range(c0, c0 + w):
                for p in range(128):
                    nd = nodes[p]
                    if nd >= 0 and d < starts[nd + 1] - starts[nd]:
                        s = sk[starts[nd] + d]
                        s1[j] = s % CH1
                        s2[j] = (s // NSH % 2) * NSHP + s % NSH
                    j += 1
        slot1w = _wrap_idx(s1)
        slot2w = _wrap_idx(s2)

        # ad idx: per (k, gridtile) 128 local dst ids (pad -> 0)
        adix = []
        for k in range(NCH):
            nd = slot_nodes[k]
            adix.append(np.where(nd >= 0, nd, 0))
        adw = _wrap_idx(np.concatenate(adix)) if stream_len else None

        # merge idx: per chunk, per natural node (padded to NSHP): grid row or zero-row
        mrg = []
        for k in range(NCH):
            deg = pc[c][k][0]
            pos = np.full(NSHP, grid_rows[k], np.int64)  # zero row
            nd = slot_nodes[k]
            real = nd >= 0
            pos[nd[real]] = np.nonzero(real)[0]
            mrg.append(pos)
        mrgw = _wrap_idx(np.concatenate(mrg))

        in_maps.append(dict(
            xT=xT, Waug1=Waug1, Waug2=Waug2.astype(BF),
            b1rep=b1rep, b2rep=b2rep, pad1=pad1, pad2=pad2, z256=z256,
            slot1w=slot1w, slot2w=slot2w, adw=adw, mrgw=mrgw,
        ))
    meta = dict(T=T, calls=calls, rowbase=rowbase, grid_rows=grid_rows,
                stream_len=stream_len)
    return in_maps, meta


def vap(t, off, dims):
    a = t[:]
    return bass.AP(a.tensor, a.offset + off, [list(a.ap[0])] + [list(d) for d in dims])


def build_nc(meta):
    T = meta["T"]
    grid_rows = meta["grid_rows"]
    stream_len = meta["stream_len"]
    SW = stream_len // 16
    ADL = sum(grid_rows)
    AW = ADL // 16
    MW = (NCH * NSHP) // 16
    XDT = BF16 if XBF16 else F32

    nc = bacc.Bacc("TRN2", target_bir_lowering=False, num_swdge_queues=NQ,
                   dynamic_dma_scratch_size=SCRATCH)
    dp = nc.declare_dram_parameter
    xT = dp("xT", [IN_DIM, N], XDT, isOutput=False)
    Waug1 = dp("Waug1", [128, 136], F32, isOutput=False)
    Waug2 = dp("Waug2", [32, 72], BF16, isOutput=False)
    b1rep = dp("b1rep", [128, HID], F32, isOutput=False)
    b2rep = dp("b2rep", [128, OUT_DIM], F32, isOutput=False)
    pad1 = dp("pad1", [1, 256], BF16, isOutput=False)
    pad2 = dp("pad2", [1, 128], BF16, isOutput=False)
    z256 = dp("z256", [1, 256], BF16, isOutput=False)
    slot1w = dp("slot1w", [128, SW], I16, isOutput=False)
    slot2w = dp("slot2w", [128, SW], I16, isOutput=False)
    adw = dp("adw", [128, AW], I16, isOutput=False)
    mrgw = dp("mrgw", [128, MW], I16, isOutput=False)
    out2 = dp("out2", [NSHP, OUT_DIM], F32, isOutput=True)

    table1 = nc.dram_tensor("table1", [NCH * CH1R, 256], BF16)
    table2 = nc.dram_tensor("table2", [NCH * CH2R, 128], BF16)
    ad1nat = nc.dram_tensor("ad1nat", [N, 4], F32)
    ad2nat = nc.dram_tensor("ad2nat", [R2, 4], F32)
    ad1c = nc.dram_tensor("ad1c", [NSHP, 64], F32)
    ad2c = nc.dram_tensor("ad2c", [NSHP, 64], F32)
    stg1 = [nc.dram_tensor(f"stg1_{k}", [grid_rows[k] + 1, 256], BF16)
            for k in range(NCH)]
    stg2 = [nc.dram_tensor(f"stg2_{k}", [grid_rows[k] + 1, 128], BF16)
            for k in range(NCH)]
    h1T_sh = nc.dram_tensor("h1T_sh", [32, NSHP], BF16)
    h1T_all = nc.dram_tensor("h1T_all", [NCORES, 32, NSHP], BF16,
                             addr_space="Shared")

    qn = [0]
    def nextq():
        qn[0] = (qn[0] + 1) % NQ
        return qn[0]

    with tile.TileContext(nc) as tc:
        nc.gpsimd.load_library(mlp_lib)

        # ---------- consts / pads ----------
        with tc.tile_pool(name="konst", bufs=1) as kp:
            w1sb = kp.tile([128, 136], XDT)
            if XBF16:
                w1f = kp.tile([128, 136], F32)
                nc.sync.dma_start(out=w1f[:], in_=Waug1[:, :])
                nc.vector.tensor_copy(out=w1sb[:], in_=w1f[:])
            else:
                nc.sync.dma_start(out=w1sb[:], in_=Waug1[:, :])
            w2sb = kp.tile([32, 72], BF16)
            nc.sync.dma_start(out=w2sb[:], in_=Waug2[:, :])
            b1sb = kp.tile([128, HID], F32)
            nc.sync.dma_start(out=b1sb[:], in_=b1rep[:, :])
            b2sb = kp.tile([128, OUT_DIM], F32)
            nc.sync.dma_start(out=b2sb[:], in_=b2rep[:, :])
            for k in range(NCH):
                nc.sync.dma_start(out=table1[k * CH1R + CH1, :], in_=pad1[0, :])
                nc.sync.dma_start(out=table2[k * CH2R + CH2, :], in_=pad2[0, :])
                nc.sync.dma_start(out=stg1[k][grid_rows[k], :], in_=z256[0, :])
                nc.sync.dma_start(out=stg2[k][grid_rows[k], :], in_=z256[0, :128])

            # ---------- dense1 ----------
            with nc.named_scope("dense1"):
                _dense1(nc, tc, xT, w1sb, table1, ad1nat, XDT)

            # ---------- repack ad1: own 12500 rows -> ad1c [NSHP, 64] ----------
            pid = nc.gpsimd.partition_id()
            with nc.named_scope("repack1"), tc.tile_pool(name="rp", bufs=2) as rp:
                adt = rp.tile([128, 98 * 4], F32, tag="adt")
                nc.gpsimd.memset(adt[:], 0.0)
                base = pid * NSH
                nc.gpsimd.dma_start(
                    out=vap(adt, 0, [[4, 97], [1, 4]]),
                    in_=ad1nat[bass.ds(base, 12416), :]
                        .rearrange("(t p) f -> p t f", p=128))
                nc.gpsimd.dma_start(
                    out=bass.AP(adt[:].tensor, adt[:].offset + 97 * 4,
                                [[list(adt[:].ap[0])[0], 84], [1, 4]]),
                    in_=ad1nat[bass.ds(base + 12416, 84), :])
                nc.sync.dma_start(
                    out=bass.AP(ad1c[:, :].tensor, 0,
                                [[64, 128], [64 * 128, 98], [1, 4]]),
                    in_=adt[:])

            # ---------- edge pass L1 ----------
            with nc.named_scope("edge1"):
                _edge_pass(nc, tc, meta, layer=1, slotw=slot1w, adw=adw,
                           table=table1, stg=stg1, ad_core=ad1c, nextq=nextq)

            # ---------- merge L1 -> h1T ----------
            with nc.named_scope("merge1"):
                _merge_pass(nc, tc, meta, layer=1, mrgw=mrgw, stg=stg1,
                            bsb=b1sb, out2=None, h1T_sh=h1T_sh, nextq=nextq)

            # ---------- allgather ----------
            with nc.named_scope("allgather"):
                nc.gpsimd.collective_compute(
                    "AllGather", AluOp.bypass,
                    replica_groups=[list(range(NCORES))],
                    ins=[h1T_sh[:, :]], outs=[h1T_all[:, :, :]])

            # ---------- dense2 ----------
            with nc.named_scope("dense2"):
                _dense2(nc, tc, h1T_all, w2sb, table2, ad2nat)

            # ---------- repack ad2 ----------
            with nc.named_scope("repack2"), tc.tile_pool(name="rp2", bufs=2) as rp:
                adt = rp.tile([128, 98 * 4], F32, tag="adt2")
                base2 = pid * NSHP
                nc.gpsimd.dma_start(
                    out=vap(adt, 0, [[4, 98], [1, 4]]),
                    in_=ad2nat[bass.ds(base2, NSHP), :]
                        .rearrange("(t p) f -> p t f", p=128))
                nc.sync.dma_start(
                    out=bass.AP(ad2c[:, :].tensor, 0,
                                [[64, 128], [64 * 128, 98], [1, 4]]),
                    in_=adt[:])

            # ---------- edge pass L2 ----------
            with nc.named_scope("edge2"):
                _edge_pass(nc, tc, meta, layer=2, slotw=slot2w, adw=adw,
                           table=table2, stg=stg2, ad_core=ad2c, nextq=nextq)

            # ---------- merge L2 -> out2 ----------
            with nc.named_scope("merge2"):
                _merge_pass(nc, tc, meta, layer=2, mrgw=mrgw, stg=stg2,
                            bsb=b2sb, out2=out2, h1T_sh=None, nextq=nextq)

    nc.finalize()
    return nc


def _row_splits(n0, nn, chunk, chunkr):
    """Split node range [n0, n0+nn) at `chunk` boundaries; yield
    (node_start, count, table_row_start)."""
    a = n0
    while a < n0 + nn:
        k = a // chunk
        b = min(n0 + nn, (k + 1) * chunk)
        yield a, b - a, k * chunkr + (a - k * chunk)
        a = b


def _write_rows(nc, dst_t, n0, nn, sb, slot, width, rowstride, chunk, chunkr):
    """Write nn rows (nodes n0..) from SBUF tile sb [128, BB*slot] where
    node n0+b*128+p lives at partition p, free offset b*slot, width elems.
    dst row stride `rowstride` elems, chunk-shifted table rows.
    Issued on the ACT sequencer (Sync carries the loads)."""
    for (a, cnt, r0) in _row_splits(n0, nn, chunk, chunkr):
        off = a - n0                       # row offset within block
        while cnt > 0:
            b, p = off // 128, off % 128
            if p == 0 and cnt >= 128:
                nbt = cnt // 128
                nc.scalar.dma_start(
                    out=bass.AP(dst_t[:, :].tensor, r0 * rowstride,
                                [[rowstride, 128], [rowstride * 128, nbt],
                                 [1, width]]),
                    in_=vap(sb, b * slot, [[slot, nbt], [1, width]]))
                took = nbt * 128
            else:
                take = min(cnt, 128 - p)
                ap = sb[p:p + take]
                nc.scalar.dma_start(
                    out=bass.AP(dst_t[:, :].tensor, r0 * rowstride,
                                [[rowstride, take], [1, width]]),
                    in_=bass.AP(ap.tensor, ap.offset + b * slot,
                                [list(ap.ap[0]), [1, width]]))
                took = take
            r0 += took
            off += took
            cnt -= took


BB = 16  # row-tiles per dense block


def _dense1(nc, tc, xT, w1sb, table1, ad1nat, XDT):
    with (tc.tile_pool(name="d1", bufs=3) as dpool,
          tc.tile_pool(name="d1p", bufs=2, space="PSUM") as dps):
        n0 = 0
        while n0 < N:
            nn = min(BB * 128, N - n0)
            bt = (nn + 127) // 128
            xm8 = dpool.tile([128, BB * 128], XDT, tag="xm8")
            nc.sync.dma_start(out=xm8[:, 0:nn], in_=bass.AP(
                xT[:, :].tensor, n0, [[N, 128], [1, nn]]))
            hrow8 = dpool.tile([128, BB * 132], BF16, tag="hrow8")
            adsb8 = dpool.tile([128, BB * 4], F32, tag="adsb8")
            for g0 in range(0, bt, 4):
                ng = min(4, bt - g0)
                ps = dps.tile([128, 2048], F32, tag="ps")
                for b in range(ng):
                    tb = g0 + b
                    nnb = min(128, nn - tb * 128)
                    nc.tensor.matmul(
                        out=ps[0:nnb, b * 512:b * 512 + 136],
                        lhsT=xm8[:, tb * 128:tb * 128 + nnb],
                        rhs=w1sb[:], start=True, stop=True)
                nc.vector.tensor_copy(
                    out=vap(hrow8, g0 * 132, [[132, ng], [1, 132]]),
                    in_=vap(ps, 0, [[512, ng], [1, 132]]))
                nc.vector.tensor_copy(
                    out=vap(adsb8, g0 * 4, [[4, ng], [1, 4]]),
                    in_=vap(ps, 132, [[512, ng], [1, 4]]))
            _write_rows(nc, table1, n0, nn, hrow8, 132, 132, 256, CH1, CH1R)
            _write_rows(nc, ad1nat, n0, nn, adsb8, 4, 4, 4, N, N)
            n0 += nn


def _dense2(nc, tc, h1T_all, w2sb, table2, ad2nat):
    with (tc.tile_pool(name="d2", bufs=3) as dpool,
          tc.tile_pool(name="d2p", bufs=2, space="PSUM") as dps):
        for cc in range(NCORES):
            shift = cc // 2                # table2 pad-row shift, const per cc
            q0 = 0
            while q0 < NT:
                nb = min(BB, NT - q0)
                lh8 = dpool.tile([32, BB * 128], BF16, tag="lh8")
                nc.sync.dma_start(out=lh8[:, 0:nb * 128], in_=bass.AP(
                    h1T_all[:, :, :].tensor, cc * 32 * NSHP + q0 * 128,
                    [[NSHP, 32], [1, nb * 128]]))
                h2row8 = dpool.tile([128, BB * 68], BF16, tag="h2row8")
                adsb8 = dpool.tile([128, BB * 4], F32, tag="adsb28")
                for g0 in range(0, nb, 4):
                    ng = min(4, nb - g0)
                    ps = dps.tile([128, 2048], F32, tag="ps2")
                    for b in range(ng):
                        nc.tensor.matmul(
                            out=ps[:, b * 512:b * 512 + 72],
                            lhsT=lh8[:, (g0 + b) * 128:(g0 + b + 1) * 128],
                            rhs=w2sb[:], start=True, stop=True)
                    nc.vector.tensor_copy(
                        out=vap(h2row8, g0 * 68, [[68, ng], [1, 68]]),
                        in_=vap(ps, 0, [[512, ng], [1, 68]]))
                    nc.vector.tensor_copy(
                        out=vap(adsb8, g0 * 4, [[4, ng], [1, 4]]),
                        in_=vap(ps, 68, [[512, ng], [1, 4]]))
                r0 = (cc * NT + q0) * 128 + shift
                nc.scalar.dma_start(
                    out=bass.AP(table2[:, :].tensor, r0 * 128,
                                [[128, 128], [128 * 128, nb], [1, 68]]),
                    in_=vap(h2row8, 0, [[68, nb], [1, 68]]))
                a0 = (cc * NT + q0) * 128
                nc.scalar.dma_start(
                    out=bass.AP(ad2nat[:, :].tensor, a0 * 4,
                                [[4, 128], [4 * 128, nb], [1, 4]]),
                    in_=vap(adsb8, 0, [[4, nb], [1, 4]]))
                q0 += nb


def _edge_pass(nc, tc, meta, layer, slotw, adw, table, stg, ad_core, nextq):
    rowbase = meta["rowbase"]
    grid_rows = meta["grid_rows"]
    RW = 256 if layer == 1 else 128       # table row elems (bf16)
    FD = 128 if layer == 1 else 64        # feature elems
    SW = meta["stream_len"] // 16
    AW = sum(grid_rows) // 16
    KR = CH1R if layer == 1 else CH2R

    # stream offset of each bucket's first slot (buckets are contiguous)
    bstart = {}
    off = 0
    for k in range(NCH):
        for bi, D in enumerate(BUCKETS):
            bstart[(k, bi)] = off
            off += int(meta["T"][k][bi]) * D * 128

    with (tc.tile_pool(name=f"eidx{layer}", bufs=1) as ip,
          tc.tile_pool(name=f"eg{layer}", bufs=2) as gp,
          tc.tile_pool(name=f"ea{layer}", bufs=2) as ap_pool,
          tc.tile_pool(name=f"ew{layer}", bufs=2) as wp):
        sidx = ip.tile([128, SW], I16, tag="sidx")
        nc.sync.dma_start(out=sidx[:], in_=slotw[:, :])
        aidx = ip.tile([128, AW], I16, tag="aidx")
        nc.sync.dma_start(out=aidx[:], in_=adw[:, :])

        abase = 0
        for k in range(NCH):
            for bi, D in enumerate(BUCKETS):
                Tb = int(meta["T"][k][bi])
                if Tb == 0:
                    continue
                ncols = Tb * D
                # ad gather for this bucket's grid tiles
                ADG = ap_pool.tile([128, Tb, 64], F32, tag="ADG")
                na = Tb * 128
                o = 0
                while o < na:
                    nb = min(MAXG, na - o)
                    nc.gpsimd.dma_gather(
                        ADG[:, (o // 128):(o + nb) // 128, :], ad_core[:, :],
                        aidx[:, (abase + o) // 16:(abase + o + nb) // 16],
                        nb, nb, 64, queue_num=nextq())
                    o += nb
                abase += na
                # slot gather
                G = gp.tile([128, ncols, RW], BF16, tag="G")
                off = bstart[(k, bi)]
                c = 0
                while c < ncols:
                    w = min(GW, ncols - c)
                    nc.gpsimd.dma_gather(
                        G[:, c:c + w, :], table[k * KR:(k + 1) * KR, :],
                        sidx[:, (off + 128 * c) // 16:(off + 128 * (c + w)) // 16],
                        128 * w, 128 * w, RW, queue_num=nextq())
                    c += w
                # e = exp(leaky(as + ad))
                e = wp.tile([128, ncols * 4], F32, tag="e")
                nc.vector.tensor_tensor(
                    out=e[:], in0=vap(G, FD, [[RW, ncols], [1, 4]]),
                    in1=vap(ADG, 0, [[64, Tb], [0, D], [1, 4]]),
                    op=AluOp.add)
                nc.vector.scalar_tensor_tensor(
                    out=e[:], in0=e[:], scalar=0.2, in1=e[:],
                    op0=AluOp.mult, op1=AluOp.max)
                nc.scalar.activation(out=e[:], in_=e[:], func=ActFn.Exp)
                # partial rows [featsum | denom], reduces write bf16 directly
                W = FD + 4
                so_all = wp.tile([128, Tb * W], BF16, tag="so")
                with nc.allow_low_precision(reason="stg partials are bf16"):
                    nc.vector.tensor_reduce(
                        out=vap(so_all, FD, [[W, Tb], [1, 4]]),
                        in_=vap(e, 0, [[4 * D, Tb], [1, 4], [4, D]]),
                        axis=Axis.X, op=AluOp.add)
                    for c0 in range(0, ncols, GC):
                        gc = min(GC, ncols - c0)
                        tg = gc // D
                        t0 = c0 // D
                        val = wp.tile([128, GC * FD], BF16, tag="val")
                        nc.vector.tensor_tensor(
                            out=val[:, 0:gc * FD],
                            in0=vap(G, c0 * RW, [[RW, gc], [FD // 4, 4], [1, FD // 4]]),
                            in1=vap(e, c0 * 4, [[4, gc], [1, 4], [0, FD // 4]]),
                            op=AluOp.mult)
                        nc.vector.tensor_reduce(
                            out=vap(so_all, t0 * W, [[W, tg], [1, FD]]),
                            in_=vap(val, 0, [[D * FD, tg], [1, FD], [FD, D]]),
                            axis=Axis.X, op=AluOp.add)
                rb = rowbase[(k, bi, 0)]
                nc.scalar.dma_start(
                    out=bass.AP(stg[k][:, :].tensor, rb * RW,
                                [[RW, 128], [RW * 128, Tb], [1, W]]),
                    in_=vap(so_all, 0, [[W, Tb], [1, W]]))


def _merge_pass(nc, tc, meta, layer, mrgw, stg, bsb, out2, h1T_sh, nextq):
    grid_rows = meta["grid_rows"]
    RW = 256 if layer == 1 else 128
    FD = 128 if layer == 1 else 64
    OD = HID if layer == 1 else OUT_DIM
    MW = (NCH * NSHP) // 16
    W = FD + 4

    with (tc.tile_pool(name=f"midx{layer}", bufs=1) as ip,
          tc.tile_pool(name=f"mg{layer}", bufs=2) as gp,
          tc.tile_pool(name=f"mw{layer}", bufs=2) as wp,
          tc.tile_pool(name=f"mp{layer}", bufs=2, space="PSUM") as pp):
        midx = ip.tile([128, MW], I16, tag="midx")
        nc.sync.dma_start(out=midx[:], in_=mrgw[:, :])
        if layer == 1:
            from concourse.masks import make_identity
            ident = ip.tile([128, 128], F32, tag="ident")
            make_identity(nc, ident[:])
            h1sb = ip.tile([32, NSHP], BF16, tag="h1sb")

        mt = 0
        while mt < NT:
            nb = min(MB, NT - mt)
            Gs = []
            for k in range(NCH):
                Gk = gp.tile([128, MB, RW], BF16, tag=f"MG{k}")
                for b0 in range(0, nb, MAXG // 128):
                    bn = min(MAXG // 128, nb - b0)
                    ioff = k * NSHP + (mt + b0) * 128
                    nc.gpsimd.dma_gather(
                        Gk[:, b0:b0 + bn, :], stg[k][:, :],
                        midx[:, ioff // 16:(ioff + bn * 128) // 16],
                        bn * 128, bn * 128, RW, queue_num=nextq())
                Gs.append(Gk)
            s01 = wp.tile([128, MB * W], F32, tag="s01")
            nc.vector.tensor_tensor(
                out=vap(s01, 0, [[W, nb], [1, W]]),
                in0=vap(Gs[0], 0, [[RW, nb], [1, W]]),
                in1=vap(Gs[1], 0, [[RW, nb], [1, W]]), op=AluOp.add)
            s23 = wp.tile([128, MB * W], F32, tag="s23")
            nc.vector.tensor_tensor(
                out=vap(s23, 0, [[W, nb], [1, W]]),
                in0=vap(Gs[2], 0, [[RW, nb], [1, W]]),
                in1=vap(Gs[3], 0, [[RW, nb], [1, W]]), op=AluOp.add)
            nc.vector.tensor_tensor(
                out=vap(s01, 0, [[W, nb], [1, W]]),
                in0=vap(s01, 0, [[W, nb], [1, W]]),
                in1=vap(s23, 0, [[W, nb], [1, W]]), op=AluOp.add)
            rec = wp.tile([128, MB * 4], F32, tag="rec")
            nc.vector.tensor_scalar_add(
                out=vap(rec, 0, [[4, nb], [1, 4]]),
                in0=vap(s01, FD, [[W, nb], [1, 4]]), scalar1=EPS)
            nc.vector.reciprocal(out=rec[:, 0:nb * 4], in_=rec[:, 0:nb * 4])
            nc.vector.tensor_scalar_mul(out=rec[:, 0:nb * 4],
                                        in0=rec[:, 0:nb * 4], scalar1=0.25)
            sc = wp.tile([128, MB * FD], F32, tag="sc")
            nc.vector.tensor_tensor(
                out=vap(sc, 0, [[FD, nb], [FD // 4, 4], [1, FD // 4]]),
                in0=vap(s01, 0, [[W, nb], [FD // 4, 4], [1, FD // 4]]),
                in1=vap(rec, 0, [[4, nb], [1, 4], [0, FD // 4]]),
                op=AluOp.mult)
            hs = wp.tile([128, MB * OD], F32, tag="hs")
            nc.vector.tensor_reduce(
                out=vap(hs, 0, [[OD, nb], [1, OD]]),
                in_=vap(sc, 0, [[FD, nb], [1, OD], [OD, 4]]),
                axis=Axis.X, op=AluOp.add)
            nc.vector.tensor_tensor(
                out=vap(hs, 0, [[OD, nb], [1, OD]]),
                in0=vap(hs, 0, [[OD, nb], [1, OD]]),
                in1=vap(bsb, 0, [[0, nb], [1, OD]]), op=AluOp.add)
            if layer == 1:
                nc.scalar.activation(out=hs[:, 0:nb * OD], in_=hs[:, 0:nb * OD],
                                     func=ActFn.Relu)
                for g0 in range(0, nb, 4):
                    gn = min(4, nb - g0)
                    psT = pp.tile([32, 512], F32, tag="psT")
                    for j in range(gn):
                        nc.tensor.transpose(
                            out=psT[:, j * 128:(j + 1) * 128],
                            in_=hs[:, (g0 + j) * OD:(g0 + j + 1) * OD],
                            identity=ident[:])
                    nc.vector.tensor_copy(
                        out=h1sb[:, (mt + g0) * 128:(mt + g0 + gn) * 128],
                        in_=psT[:, 0:gn * 128])
            else:
                nc.scalar.dma_start(
                    out=bass.AP(out2[:, :].tensor, mt * 128 * OD,
                                [[OD, 128], [OD * 128, nb], [1, OD]]),
                    in_=vap(hs, 0, [[OD, nb], [1, OD]]))
            mt += nb
        if layer == 1:
            nc.scalar.dma_start(out=h1T_sh[:, :], in_=h1sb[:])


_CACHE = {}


def kernel(**inputs):
    in_maps, meta = host_prep(**inputs)
    key = str(meta["T"])
    _CACHE["k"] = key
    if key not in _CACHE:
        nc = build_nc(meta)
        _CACHE[key] = (nc, make_runner(nc, NCORES))
    nc, run = _CACHE[key]
    results, best = run(in_maps, repeats=1)
    _CACHE["last_time"] = best
    out = np.empty((N, OUT_DIM), np.float32)
    for c in range(NCORES):
        out[c * NSH:(c + 1) * NSH] = results[c]["out2"][:NSH]
    return out
